# revision 1
# baseline (speedup 1.0000x reference)
"""AttentionPool2d Trainium2 kernel (8-core data parallel over batch).

Math (per batch item), exploiting that only query token 0 survives into the
output: tokens t = [mean(x); x_tokens] + pos_emb; v = t @ Wv.T + bv;
out[1:] = v[1:] @ Wc.T + bc; out[0] = softmax(q0·K/sqrt(hd)) V @ Wc.T + bc
with q0 = K = V = v (per head). So: compute vT = (Wv t.T + ...) in
[channel, token] layout, do the 1-query attention with mask matmuls,
substitute ctx into token-0 columns of vT, and run one out-projection
u.T @ Wc.T over all 50 tokens.

All matmuls fp16 (measured l2 rel err ~3e-4 per matmul on TRN2; fp32
accumulation in PSUM). pos_emb and bv are folded into a host-precomputed
vposT = (pos_emb @ Wv.T + bv).T added during the PSUM->SBUF copy.
"""

import numpy as np

import bass_rust
import concourse.bass as bass
import concourse.mybir as mybir
import concourse.tile as tile
from concourse.bass_utils import run_bass_kernel_spmd
from concourse.tile_scheduler import PROC_NAME_TO_IDX
from contextlib import ExitStack

# ---------------------------------------------------------------- constants
B, C, S = 256, 2048, 7
HW = S * S              # 49 spatial tokens
N = HW + 1              # 50 tokens incl. mean token
H, OUT = 32, 1024        # default num_heads; build is parameterized
HD = C // H
SCALE = HD ** -0.5
CORES = 8
IPC = B // CORES        # 32 items per core
GI = 8                  # items per group
G = IPC // GI           # 4 groups
TOK = IPC * N           # 1600 token columns per core
KC = C // 128           # 16 contraction chunks
JC = C // 128           # 16 output-channel chunks of v
NG = GI * N             # 400 moving columns per group
OC2 = OUT // 512        # 2 out-projection column chunks

F16 = mybir.dt.float16
F32 = mybir.dt.float32

N_PROCS = 27


# ------------------------------------------------------- tile/walrus patches
def _patched_drain_and_barrier(self, tick_clock, wait_clock):
    """Stock tail drain carries one wait per ticked proc; walrus here allows
    a single sync-wait per instruction. Funnel waits through SP nops."""
    nc = self.nc
    gc = tick_clock.global_clock
    ticks = [gc.peek_next(i) - 1 for i in range(N_PROCS)]
    live = [i for i in range(N_PROCS) if ticks[i] > 0]
    sp_clock = wait_clock.engine_clocks[PROC_NAME_TO_IDX["SP"]]
    for p in live:
        vc = bass_rust.VectorClock()
        vc.require_at_least(p, ticks[p])
        nop = nc.sync.nop(nofuse=True, hint="tail_wait_funnel")
        wait_clock.add_sem_waits(
            nop.ins, bass_rust.ScopedClock({None: vc}), cur_clock=sp_clock
        )
        sp_clock.require_at_least(None, p, ticks[p])
    drain_inst = nc.sync.drain()
    wait_clock.add_sem_waits(
        drain_inst.ins, bass_rust.ScopedClock({None: gc}), cur_clock=sp_clock
    )
    nc.all_engine_barrier()
    assert self.sems is not None
    popped = nc._tile_sem_poison_stack.pop()
    assert popped is self._sem_poison
    nc.clear_and_free_semaphores(list(self.sems.allocated().values()))
    nc.all_engine_barrier()


tile.TileContext._drain_and_barrier = _patched_drain_and_barrier


def fix_excess_waits(nc, max_waits=1):
    """Hoist excess per-instruction sync-waits onto injected same-engine
    NoOps placed immediately before the offender (engine streams run in
    basic-block order)."""
    for bb in nc.m.functions[0].blocks:
        insts = bb.instructions
        if not any(
            i.sync_info and i.sync_info.on_wait and len(i.sync_info.on_wait) > max_waits
            for i in insts
        ):
            continue
        out = []
        for inst in insts:
            si = inst.sync_info
            if si and si.on_wait and len(si.on_wait) > max_waits:
                waits = list(si.on_wait)
                extra, keep = waits[:-max_waits], waits[-max_waits:]
                for i in range(0, len(extra), max_waits):
                    chunk = extra[i : i + max_waits]
                    nop = mybir.InstNoOp(
                        name=nc.get_next_instruction_name(), ins=[], outs=[]
                    )
                    nop.engine = inst.engine
                    nop.sync_info = bass_rust.SyncInfo(on_wait=chunk, on_update=[])
                    nc.register_instruction(nop)
                    out.append(nop)
                si.on_wait = keep
            out.append(inst)
        bb.instructions = out


# ------------------------------------------------------------- kernel build
def build_kernel(reps=1, variant="full", heads=H):
    nc = bass.Bass("TRN2", target_bir_lowering=False, debug=False)

    x_d = nc.dram_tensor("x", [IPC, C, HW], F16, kind="ExternalInput")
    wv_d = nc.dram_tensor("wvT", [C, C], F16, kind="ExternalInput")
    wc_d = nc.dram_tensor("wcT", [C, OUT], F16, kind="ExternalInput")
    vpos_d = nc.dram_tensor("vposT", [128, KC * N], F32, kind="ExternalInput")
    maskT_d = nc.dram_tensor("maskT", [128, KC * heads], F16, kind="ExternalInput")
    mask2_d = nc.dram_tensor("mask2", [heads, KC * 128], F16, kind="ExternalInput")
    out_d = nc.dram_tensor("out", [IPC, N, OUT], F32, kind="ExternalOutput")
    out_flat = out_d.ap().rearrange("i n o -> (i n) o")

    with tile.TileContext(nc) as tc, ExitStack() as ctx:
        wv_pool = ctx.enter_context(tc.tile_pool(name="wv", bufs=1))
        wc_pool = ctx.enter_context(tc.tile_pool(name="wc", bufs=1))
        cpool = ctx.enter_context(tc.tile_pool(name="consts", bufs=1))
        xpool = ctx.enter_context(tc.tile_pool(name="xstage", bufs=3))
        tpool = ctx.enter_context(tc.tile_pool(name="tT", bufs=2))
        vpool = ctx.enter_context(tc.tile_pool(name="vT", bufs=1))
        apool = ctx.enter_context(tc.tile_pool(name="attn", bufs=2))
        opool = ctx.enter_context(tc.tile_pool(name="outsb", bufs=2))
        pv = ctx.enter_context(tc.tile_pool(name="pv", bufs=2, space="PSUM"))
        pS = ctx.enter_context(tc.tile_pool(name="pS", bufs=2, space="PSUM"))
        pA = ctx.enter_context(tc.tile_pool(name="pA", bufs=2, space="PSUM"))
        po = ctx.enter_context(tc.tile_pool(name="po", bufs=2, space="PSUM"))

        # ---- resident weights/constants
        wv_sb = []
        for kc in range(KC):
            w = wv_pool.tile([128, C], F16, name=f"wv{kc}", tag=f"wv{kc}")
            nc.sync.dma_start(w[:], wv_d.ap()[kc * 128 : (kc + 1) * 128, :])
            wv_sb.append(w)
        wc_sb = []
        for kc in range(KC):
            w = wc_pool.tile([128, OUT], F16, name=f"wc{kc}", tag=f"wc{kc}")
            nc.sync.dma_start(w[:], wc_d.ap()[kc * 128 : (kc + 1) * 128, :])
            wc_sb.append(w)
        vpos_sb = cpool.tile([128, KC * N], F32, name="vpos")
        nc.sync.dma_start(vpos_sb[:], vpos_d.ap())
        maskT_sb = cpool.tile([128, KC * heads], F16, name="maskT")
        nc.sync.dma_start(maskT_sb[:], maskT_d.ap())
        mask2_sb = cpool.tile([heads, KC * 128], F16, name="mask2")
        nc.sync.dma_start(mask2_sb[:], mask2_d.ap())

        # vT_all[jb]: [128, TOK] fp16, channel block jb x all token columns
        vT = []
        for jb in range(JC):
            v = vpool.tile([128, TOK], F16, name=f"vT{jb}", tag=f"vT{jb}")
            vT.append(v)

        def work():
            body(nc, tc, x_d, out_flat, wv_sb, wc_sb, vpos_sb, maskT_sb,
                 mask2_sb, vT, tpool, xpool, apool, opool, pv, pS, pA, po,
                 variant, heads)

        if reps == 1:
            work()
        else:
            with tc.For_i(0, reps, 1):
                work()

    fix_excess_waits(nc)
    return nc


def body(nc, tc, x_d, out_flat, wv_sb, wc_sb, vpos_sb, maskT_sb, mask2_sb,
         vT, tpool, xpool, apool, opool, pv, pS, pA, po, variant="full",
         heads=H):
    scale = (C // heads) ** -0.5
    vpos3 = vpos_sb[:].rearrange("p (k n) -> p k n", k=KC)

    def build_tT(g):
        # tT layout: [128, KC*(GI*N)] fp16 -- kc-major blocks of 400 cols so
        # the matmul moving operand is contiguous: col = kc*400 + it*50 + n
        tT = tpool.tile([128, KC * GI * N], F16, name="tT", tag="tT")
        tT4 = tT[:].rearrange("p (k i n) -> p k i n", k=KC, i=GI)
        for it in range(GI):
            gi = g * GI + it
            xs = xpool.tile([128, KC * HW], F16, name="xs", tag="xs")
            xs3 = xs[:].rearrange("p (k n) -> p k n", k=KC)
            # x[gi] is [C, HW] row-major; channel chunk kc -> partition p
            nc.sync.dma_start(
                xs[:],
                x_d.ap()[gi].rearrange("(k p) n -> p k n", p=128),
            )
            # spatial tokens, cast to fp16
            nc.scalar.activation(
                tT4[:, :, it, 1:N],
                xs3,
                mybir.ActivationFunctionType.Copy,
            )
            # mean token: reduce over the 49 spatial positions
            xsum = xpool.tile([128, KC], F32, name="xsum", tag="xsum")
            nc.vector.reduce_sum(xsum[:], xs3, axis=mybir.AxisListType.X)
            nc.scalar.activation(
                tT4[:, :, it, 0],
                xsum[:],
                mybir.ActivationFunctionType.Copy,
                scale=1.0 / HW,
            )
        return tT

    def vproj(g, tT):
        g0 = g * NG
        for jb in range(JC):
            psum = pv.tile([128, NG], F32, name="pvt", tag="pvt")
            for kc in range(KC):
                nc.tensor.matmul(
                    psum[:],
                    wv_sb[kc][:, jb * 128 : (jb + 1) * 128],
                    tT[:, kc * NG : (kc + 1) * NG],
                    start=(kc == 0),
                    stop=(kc == KC - 1),
                )
            # add vposT (same 50-col pattern for every item) + fp16 round
            nc.vector.tensor_add(
                vT[jb][:, g0 : g0 + NG].rearrange("p (i n) -> p i n", i=GI),
                psum[:].rearrange("p (i n) -> p i n", i=GI),
                vpos3[:, jb : jb + 1, :].broadcast_to((128, GI, N)),
            )

    def attention(g):
        g0 = g * NG
        # P[jb][c, it*50+m] = vT[c, it*50+m] * vT[c, it*50+0]
        psum_S = pS.tile([heads, NG], F32, name="psS", tag="psS")
        for jb in range(JC):
            vg3 = vT[jb][:, g0 : g0 + NG].rearrange("p (i n) -> p i n", i=GI)
            p = apool.tile([128, NG], F16, name="pprod", tag="pprod")
            nc.vector.tensor_mul(
                p[:].rearrange("p (i n) -> p i n", i=GI),
                vg3,
                vg3[:, :, 0:1].broadcast_to((128, GI, N)),
            )
            nc.tensor.matmul(
                psum_S[:],
                maskT_sb[:, jb * heads : (jb + 1) * heads],
                p[:],
                start=(jb == 0),
                stop=(jb == JC - 1),
            )
        # E = exp(S * scale), denominators per item block, A = E/D
        e_sb = apool.tile([heads, NG], F32, name="esb", tag="esb")
        nc.scalar.activation(
            e_sb[:], psum_S[:], mybir.ActivationFunctionType.Exp, scale=scale
        )
        d_sb = apool.tile([heads, GI], F32, name="dsb", tag="dsb")
        nc.vector.reduce_sum(
            d_sb[:],
            e_sb[:].rearrange("p (i n) -> p i n", i=GI),
            axis=mybir.AxisListType.X,
        )
        r_sb = apool.tile([heads, GI], F32, name="rsb", tag="rsb")
        nc.vector.reciprocal(r_sb[:], d_sb[:])
        a_sb = apool.tile([heads, NG], F16, name="asb", tag="asb")
        nc.vector.tensor_mul(
            a_sb[:].rearrange("p (i n) -> p i n", i=GI),
            e_sb[:].rearrange("p (i n) -> p i n", i=GI),
            r_sb[:].rearrange("p (i o) -> p i o", o=1).broadcast_to((heads, GI, N)),
        )
        # ctx[c] = sum_m A[head(c), m] vT[c, m]; write into token-0 cols
        for jb in range(JC):
            psum_a = pA.tile([128, NG], F32, name="psA", tag="psA")
            nc.tensor.matmul(
                psum_a[:],
                mask2_sb[:, jb * 128 : (jb + 1) * 128],
                a_sb[:],
                start=True,
                stop=True,
            )
            p2 = apool.tile([128, NG], F32, name="p2", tag="p2")
            nc.vector.tensor_mul(p2[:], psum_a[:], vT[jb][:, g0 : g0 + NG])
            ctx8 = apool.tile([128, GI], F32, name="ctx8", tag="ctx8")
            nc.vector.reduce_sum(
                ctx8[:],
                p2[:].rearrange("p (i n) -> p i n", i=GI),
                axis=mybir.AxisListType.X,
            )
            nc.scalar.activation(
                vT[jb][:, g0 : g0 + NG].rearrange("p (i n) -> p i n", i=GI)[
                    :, :, 0
                ],
                ctx8[:],
                mybir.ActivationFunctionType.Copy,
            )

    def outproj(mtiles):
        # out[tok, :] = uT.T @ WcT; token-stationary, 128 tokens per tile
        for m0, mw in mtiles:
            osb = opool.tile([128, OUT], F32, name="osb", tag="osb")
            for oc in range(OC2):
                psum = po.tile([128, 512], F32, name="pso", tag="pso")
                for kc in range(KC):
                    nc.tensor.matmul(
                        psum[:mw, :],
                        vT[kc][:, m0 : m0 + mw],
                        wc_sb[kc][:, oc * 512 : (oc + 1) * 512],
                        start=(kc == 0),
                        stop=(kc == KC - 1),
                    )
                nc.vector.tensor_copy(
                    osb[:mw, oc * 512 : (oc + 1) * 512], psum[:mw, :]
                )
            nc.sync.dma_start(out_flat[m0 : m0 + mw, :], osb[:mw, :])

    # Software-pipelined schedule: attention(g) PE work hides under
    # vproj(g+1); out-projection for tokens of groups 0..2 starts before
    # the last group attention completes.
    mtiles = [(m, min(128, TOK - m)) for m in range(0, TOK, 128)]
    early = [mt for mt in mtiles if mt[0] + mt[1] <= 3 * NG]
    late = [mt for mt in mtiles if mt[0] + mt[1] > 3 * NG]

    if variant == "full":
        tT0 = build_tT(0)
        vproj(0, tT0)
        tT1 = build_tT(1)
        vproj(1, tT1)
        attention(0)
        tT2 = build_tT(2)
        vproj(2, tT2)
        attention(1)
        tT3 = build_tT(3)
        vproj(3, tT3)
        attention(2)
        outproj(early)
        attention(3)
        outproj(late)
    elif variant == "vproj":
        for g in range(G):
            vproj(g, build_tT(g))
    elif variant == "vproj+attn":
        tT0 = build_tT(0)
        vproj(0, tT0)
        tT1 = build_tT(1)
        vproj(1, tT1)
        attention(0)
        tT2 = build_tT(2)
        vproj(2, tT2)
        attention(1)
        tT3 = build_tT(3)
        vproj(3, tT3)
        attention(2)
        attention(3)
    elif variant == "outproj":
        outproj(early)
        outproj(late)
    elif variant == "tT":
        for g in range(G):
            build_tT(g)


_NC_CACHE = {}
_RUN_CACHE = {}


def _get_nc(heads):
    if heads not in _NC_CACHE:
        _NC_CACHE[heads] = build_kernel(heads=heads)
    return _NC_CACHE[heads]


def _run(nc, in_maps):
    """run_bass_kernel_spmd equivalent (axon/PJRT path) with: the jitted
    executable cached across calls, weight-like inputs passed replicated
    (uploaded once, not 8x), and donated output buffers created on device
    (no zero upload)."""
    import jax
    import jax.numpy as jnp
    import numpy as _np
    from jax.sharding import Mesh, PartitionSpec, NamedSharding
    from jax.experimental.shard_map import shard_map
    import concourse.mybir as mb
    from concourse import bass2jax as b2j

    # inputs where every core got the identical array object -> replicated
    replicated = {
        nm
        for nm in in_maps[0]
        if all(m[nm] is in_maps[0][nm] for m in in_maps)
    }

    key = id(nc)
    if key not in _RUN_CACHE:
        b2j.install_neuronx_cc_hook()
        in_names, out_names, out_avals = [], [], []
        partition_name = (
            nc.partition_id_tensor.name if nc.partition_id_tensor else None
        )
        for alloc in nc.m.functions[0].allocations:
            if not isinstance(alloc, mb.MemoryLocationSet):
                continue
            name = alloc.memorylocations[0].name
            if alloc.kind == "ExternalInput":
                if name != partition_name:
                    in_names.append(name)
            elif alloc.kind == "ExternalOutput":
                shape = tuple(alloc.tensor_shape)
                dtype = mb.dt.np(alloc.dtype)
                out_names.append(name)
                out_avals.append(jax.core.ShapedArray(shape, dtype))
        n_params = len(in_names)
        n_outs = len(out_avals)
        all_names = list(in_names) + list(out_names)
        if partition_name is not None:
            all_names.append(partition_name)
        donate = tuple(range(n_params, n_params + n_outs))

        def _body(*args):
            operands = list(args)
            if partition_name is not None:
                operands.append(b2j.partition_id_tensor())
            outs = b2j._bass_exec_p.bind(
                *operands,
                out_avals=tuple(out_avals),
                in_names=tuple(all_names),
                out_names=tuple(out_names),
                lowering_input_output_aliases=(),
                sim_require_finite=True,
                sim_require_nnan=True,
                nc=nc,
            )
            return tuple(outs)

        devices = jax.devices()[:CORES]
        mesh = Mesh(_np.asarray(devices), ("core",))
        in_specs = tuple(
            PartitionSpec() if nm in replicated else PartitionSpec("core")
            for nm in in_names
        ) + (PartitionSpec("core"),) * n_outs
        out_specs = (PartitionSpec("core"),) * n_outs
        sharded = jax.jit(
            shard_map(
                _body, mesh=mesh, in_specs=in_specs, out_specs=out_specs,
                check_rep=False,
            ),
            donate_argnums=donate,
            keep_unused=True,
        )
        zeros_fns = [
            jax.jit(
                (lambda shape, dtype: lambda: jnp.zeros(shape, dtype))(
                    (CORES * av.shape[0], *av.shape[1:]), av.dtype
                ),
                out_shardings=NamedSharding(mesh, PartitionSpec("core")),
            )
            for av in out_avals
        ]
        _RUN_CACHE[key] = (
            sharded, in_names, out_names, out_avals, zeros_fns, replicated
        )

    sharded, in_names, out_names, out_avals, zeros_fns, replicated_c = (
        _RUN_CACHE[key]
    )
    assert replicated == replicated_c, "replication pattern changed"
    args = [
        _np.asarray(in_maps[0][nm])
        if nm in replicated
        else _np.concatenate([_np.asarray(m[nm]) for m in in_maps], axis=0)
        for nm in in_names
    ]
    dev_zeros = [f() for f in zeros_fns]
    out_arrs = sharded(*args, *dev_zeros)
    return [
        {
            nm: _np.asarray(out_arrs[i]).reshape(CORES, *out_avals[i].shape)[c]
            for i, nm in enumerate(out_names)
        }
        for c in range(CORES)
    ]


# ---------------------------------------------------------------- host side
def make_in_maps(inputs, heads=H):

    x = np.asarray(inputs["x"], np.float32)
    pos_emb = np.asarray(inputs["pos_emb"], np.float32)
    Wv = np.asarray(inputs["Wv"], np.float32)
    bv = np.asarray(inputs["bv"], np.float32)
    Wc = np.asarray(inputs["Wc"], np.float32)
    bc = np.asarray(inputs["bc"], np.float32)
    num_heads = int(np.asarray(inputs["num_heads"]))
    assert num_heads == heads and x.shape == (B, C, S, S)
    assert 1 <= heads <= 128 and C % heads == 0

    wvT = np.ascontiguousarray(Wv.T).astype(np.float16)
    wcT = np.ascontiguousarray(Wc.T).astype(np.float16)

    # vposT[128, kc*50 + n] = (pos_emb @ Wv.T + bv).T chunk-tiled
    vpos = (pos_emb @ Wv.T + bv).astype(np.float32)  # [N, C]
    vposT = np.empty((128, KC * N), np.float32)
    for kc in range(KC):
        vposT[:, kc * N : (kc + 1) * N] = vpos[:, kc * 128 : (kc + 1) * 128].T

    # maskT[p, kc*heads + h] = 1 if channel kc*128+p belongs to head h
    head_of = np.arange(C) // (C // heads)
    maskT = np.zeros((128, KC * heads), np.float16)
    mask2 = np.zeros((heads, KC * 128), np.float16)
    for kc in range(KC):
        for p in range(128):
            h = head_of[kc * 128 + p]
            maskT[p, kc * heads + h] = 1.0
            mask2[h, kc * 128 + p] = 1.0

    xr16 = np.ascontiguousarray(x.reshape(B, C, HW).astype(np.float16))
    in_maps = []
    for core in range(CORES):
        in_maps.append(
            {
                "x": xr16[core * IPC : (core + 1) * IPC],
                "wvT": wvT,
                "wcT": wcT,
                "vposT": vposT,
                "maskT": maskT,
                "mask2": mask2,
            }
        )

    return in_maps


def kernel(**inputs):
    from concourse._compat import axon_active

    heads = int(np.asarray(inputs["num_heads"]))
    in_maps = make_in_maps(inputs, heads)
    nc = _get_nc(heads)
    if axon_active():
        results = _run(nc, in_maps)
    else:
        results = run_bass_kernel_spmd(nc, in_maps, list(range(CORES))).results
    out = np.concatenate([results[i]["out"] for i in range(CORES)], axis=0)
    out = np.ascontiguousarray(out, dtype=np.float32)
    bc = np.asarray(inputs["bc"], np.float32)
    if bc.any():
        out = out + bc[None, None, :]
    return out



# revision 6
# speedup vs baseline: 1.2932x; 1.2932x over previous
"""AttentionPool2d Trainium2 kernel (8-core data parallel over batch).

Math (per batch item), exploiting that only query token 0 survives into the
output: tokens t = [mean(x); x_tokens] + pos_emb; v = t @ Wv.T + bv;
out[1:] = v[1:] @ Wc.T + bc; out[0] = softmax(q0.K/sqrt(hd)) V @ Wc.T + bc
with q0 = K = V = v (per head).

Split into two precision domains:
 - tokens 1..49 (98% of the output mass) bypass v entirely:
   out[n] = x_n @ W2.T + pconst[n], W2 = Wc @ Wv (host-precomputed, fp16
   matmul on device). pconst[n] = pos_n @ W2.T + bv @ Wc.T is folded into
   the same PSUM accumulation via a one-hot 17th matmul.
 - token 0 goes through attention, where ~4% relative error is invisible
   in the full-output l2 (weight ~1/50): v is computed with fp8-e4m3
   DoubleRow matmuls (2x PE throughput; scales 32*t and 64*Wv keep
   everything in e4m3 normal range, TRN max 240), attention runs on
   vT' = 2048*v fp16, and out0 = u @ Wc with u,Wc in fp8.

Measured end-to-end l2 vs reference ~8e-4 (budget 2e-2).
"""

import numpy as np

import bass_rust
import concourse.bass as bass
import concourse.mybir as mybir
import concourse.tile as tile
from concourse.bass_utils import run_bass_kernel_spmd
from concourse.tile_scheduler import PROC_NAME_TO_IDX
from contextlib import ExitStack

# ---------------------------------------------------------------- constants
B, C, S = 256, 2048, 7
HW = S * S              # 49 spatial tokens
N = HW + 1              # 50 tokens incl. mean token
H, OUT = 32, 1024       # default num_heads; build is parameterized
HD = C // H
CORES = 8
IPC = B // CORES        # 32 items per core
GI = 8                  # items per group
G = IPC // GI           # 4 groups
NG = GI * N             # 400 moving columns per group
KC = C // 128           # 16 contraction chunks
KC2 = KC // 2           # 8 fp8 DoubleRow super-chunks
JC = C // 128           # 16 output-channel chunks of v
XTOK = IPC * HW         # 1568 spatial tokens per core (x-path)
NT = (XTOK + 127) // 128  # 13 x-path token tiles
OC2 = OUT // 512        # 2 out-projection column chunks

# fp8 scaling: tT8 = 32*t, wv8 = 64*Wv  =>  psum = 2048*(t@Wv.T)
SV = 2048.0             # vT' = SV * v
SU = 32.0               # uT = SU * ctx
SW = 64.0               # wc8 = SW * Wc

F8 = mybir.dt.float8e4
F16 = mybir.dt.float16
F32 = mybir.dt.float32

N_PROCS = 27


# ------------------------------------------------------- tile/walrus patches
def _patched_drain_and_barrier(self, tick_clock, wait_clock):
    """Stock tail drain carries one wait per ticked proc; walrus here allows
    a single sync-wait per instruction. Funnel waits through SP nops."""
    nc = self.nc
    gc = tick_clock.global_clock
    ticks = [gc.peek_next(i) - 1 for i in range(N_PROCS)]
    live = [i for i in range(N_PROCS) if ticks[i] > 0]
    sp_clock = wait_clock.engine_clocks[PROC_NAME_TO_IDX["SP"]]
    for p in live:
        vc = bass_rust.VectorClock()
        vc.require_at_least(p, ticks[p])
        nop = nc.sync.nop(nofuse=True, hint="tail_wait_funnel")
        wait_clock.add_sem_waits(
            nop.ins, bass_rust.ScopedClock({None: vc}), cur_clock=sp_clock
        )
        sp_clock.require_at_least(None, p, ticks[p])
    drain_inst = nc.sync.drain()
    wait_clock.add_sem_waits(
        drain_inst.ins, bass_rust.ScopedClock({None: gc}), cur_clock=sp_clock
    )
    nc.all_engine_barrier()
    assert self.sems is not None
    popped = nc._tile_sem_poison_stack.pop()
    assert popped is self._sem_poison
    nc.clear_and_free_semaphores(list(self.sems.allocated().values()))
    nc.all_engine_barrier()


tile.TileContext._drain_and_barrier = _patched_drain_and_barrier


def fix_excess_waits(nc, max_waits=1):
    """Hoist excess per-instruction sync-waits onto injected same-engine
    NoOps placed immediately before the offender (engine streams run in
    basic-block order)."""
    for bb in nc.m.functions[0].blocks:
        insts = bb.instructions
        if not any(
            i.sync_info and i.sync_info.on_wait and len(i.sync_info.on_wait) > max_waits
            for i in insts
        ):
            continue
        out = []
        for inst in insts:
            si = inst.sync_info
            if si and si.on_wait and len(si.on_wait) > max_waits:
                waits = list(si.on_wait)
                extra, keep = waits[:-max_waits], waits[-max_waits:]
                for i in range(0, len(extra), max_waits):
                    chunk = extra[i : i + max_waits]
                    nop = mybir.InstNoOp(
                        name=nc.get_next_instruction_name(), ins=[], outs=[]
                    )
                    nop.engine = inst.engine
                    nop.sync_info = bass_rust.SyncInfo(on_wait=chunk, on_update=[])
                    nc.register_instruction(nop)
                    out.append(nop)
                si.on_wait = keep
            out.append(inst)
        bb.instructions = out


# ------------------------------------------------------------- kernel build
def build_kernel(reps=1, variant="full", heads=H):
    nc = bass.Bass("TRN2", target_bir_lowering=False, debug=False)

    x_d = nc.dram_tensor("x", [IPC, C, HW], F16, kind="ExternalInput")
    wv8_d = nc.dram_tensor("wv8", [C, C], F8, kind="ExternalInput")
    w2_d = nc.dram_tensor("w2T", [C, OUT], F16, kind="ExternalInput")
    wc8_d = nc.dram_tensor("wc8", [C, OUT], F8, kind="ExternalInput")
    vpos_d = nc.dram_tensor("vposT", [128, KC * N], F32, kind="ExternalInput")
    maskT_d = nc.dram_tensor("maskT", [128, KC * heads], F16, kind="ExternalInput")
    mask2_d = nc.dram_tensor("mask2", [heads, KC * 128], F16, kind="ExternalInput")
    oneh_d = nc.dram_tensor("oneh", [128, NT * 128], F16, kind="ExternalInput")
    pcm_d = nc.dram_tensor("pcm", [128, OUT], F16, kind="ExternalInput")
    out_d = nc.dram_tensor("out", [IPC, N, OUT], F32, kind="ExternalOutput")

    with tile.TileContext(nc) as tc, ExitStack() as ctx:
        wv_pool = ctx.enter_context(tc.tile_pool(name="wv", bufs=1))
        w2_pool = ctx.enter_context(tc.tile_pool(name="w2", bufs=1))
        wc_pool = ctx.enter_context(tc.tile_pool(name="wc", bufs=1))
        cpool = ctx.enter_context(tc.tile_pool(name="consts", bufs=1))
        xpool = ctx.enter_context(tc.tile_pool(name="xT", bufs=1))
        spool = ctx.enter_context(tc.tile_pool(name="small", bufs=2))
        tpool = ctx.enter_context(tc.tile_pool(name="tT8", bufs=2))
        vpool = ctx.enter_context(tc.tile_pool(name="vT", bufs=2))
        apool = ctx.enter_context(tc.tile_pool(name="attn", bufs=2))
        opool = ctx.enter_context(tc.tile_pool(name="outsb", bufs=2))
        upool = ctx.enter_context(tc.tile_pool(name="uT", bufs=1))
        pv = ctx.enter_context(tc.tile_pool(name="pv", bufs=2, space="PSUM"))
        pS = ctx.enter_context(tc.tile_pool(name="pS", bufs=2, space="PSUM"))
        pA = ctx.enter_context(tc.tile_pool(name="pA", bufs=2, space="PSUM"))
        po = ctx.enter_context(tc.tile_pool(name="po", bufs=2, space="PSUM"))

        # ---- resident weights/constants (loaded outside the rep loop)
        wv8_sb = wv_pool.tile([128, KC * C], F8, name="wv8")
        for kc in range(KC):
            nc.sync.dma_start(
                wv8_sb[:, kc * C : (kc + 1) * C],
                wv8_d.ap()[kc * 128 : (kc + 1) * 128, :],
            )
        w2_sb, wc8_sb = [], []
        for kc in range(KC):
            w = w2_pool.tile([128, OUT], F16, name=f"w2{kc}", tag=f"w2{kc}")
            nc.sync.dma_start(w[:], w2_d.ap()[kc * 128 : (kc + 1) * 128, :])
            w2_sb.append(w)
            w8 = wc_pool.tile([128, OUT], F8, name=f"wc{kc}", tag=f"wc{kc}")
            nc.sync.dma_start(w8[:], wc8_d.ap()[kc * 128 : (kc + 1) * 128, :])
            wc8_sb.append(w8)
        vpos_sb = cpool.tile([128, KC * N], F32, name="vpos")
        nc.sync.dma_start(vpos_sb[:], vpos_d.ap())
        maskT_sb = cpool.tile([128, KC * heads], F16, name="maskT")
        nc.sync.dma_start(maskT_sb[:], maskT_d.ap())
        mask2_sb = cpool.tile([heads, KC * 128], F16, name="mask2")
        nc.sync.dma_start(mask2_sb[:], mask2_d.ap())
        oneh_sb = cpool.tile([128, NT * 128], F16, name="oneh")
        nc.sync.dma_start(oneh_sb[:], oneh_d.ap())
        pcm_sb = cpool.tile([128, OUT], F16, name="pcm")
        nc.sync.dma_start(pcm_sb[:], pcm_d.ap())

        # x tokens resident in [channel, kc-major global token] layout:
        # xT[p, kc, j] = x[item j//49, kc*128+p, j%49], fp16
        xT_sb = xpool.tile([128, KC * XTOK], F16, name="xTall")
        # uT[p, kc, i] = SU * ctx[item i, kc*128+p], fp8
        uT_sb = upool.tile([128, KC * IPC], F8, name="uT")

        def work():
            body(nc, tc, x_d, out_d, wv8_sb, w2_sb, wc8_sb, vpos_sb,
                 maskT_sb, mask2_sb, oneh_sb, pcm_sb, xT_sb, uT_sb,
                 spool, tpool, vpool, apool, opool, pv, pS, pA, po,
                 variant, heads)

        if reps == 1:
            work()
        else:
            with tc.For_i(0, reps, 1):
                work()

    fix_excess_waits(nc)
    return nc


def body(nc, tc, x_d, out_d, wv8_sb, w2_sb, wc8_sb, vpos_sb, maskT_sb,
         mask2_sb, oneh_sb, pcm_sb, xT_sb, uT_sb, spool, tpool, vpool,
         apool, opool, pv, pS, pA, po, variant="full", heads=H):
    out_flat = out_d.ap().rearrange("i n o -> (i n) o")
    scale_exp = float((C // heads) ** -0.5 / 64.0)
    wv8_v = wv8_sb[:].rearrange("p (k c) -> p k c", k=KC)
    xT_v = xT_sb[:].rearrange("p (k j) -> p k j", k=KC)
    uT_v = uT_sb[:].rearrange("p (k i) -> p k i", k=KC)
    vpos3 = vpos_sb[:].rearrange("p (k n) -> p k n", k=KC)

    def build_tT8(g):
        # tT8 layout: [128, KC*(GI*N)] fp8 = 32*t, kc-major so the DoubleRow
        # moving operand spans 2 adjacent kc subtiles: [p, 2, 400]
        tT8 = tpool.tile([128, KC * GI * N], F8, name="tT8", tag="tT8")
        t4 = tT8[:].rearrange("p (k i n) -> p k i n", k=KC, i=GI)
        for it in range(GI):
            gi = g * GI + it
            dst = xT_v[:, :, gi * HW : (gi + 1) * HW]
            nc.sync.dma_start(
                dst, x_d.ap()[gi].rearrange("(k p) n -> p k n", p=128)
            )
            # spatial tokens: fp8(32 * x)
            nc.scalar.activation(
                t4[:, :, it, 1:N], dst,
                mybir.ActivationFunctionType.Copy, scale=32.0,
            )
            # mean token: fp8(32/49 * sum_s x)
            xsum = spool.tile([128, KC], F32, name="xsum", tag="xsum")
            nc.vector.reduce_sum(xsum[:], dst, axis=mybir.AxisListType.X)
            nc.scalar.activation(
                t4[:, :, it, 0], xsum[:],
                mybir.ActivationFunctionType.Copy, scale=32.0 / HW,
            )
        return tT8

    def vproj(g, tT8):
        # vT' = SV * v fp16 via fp8 DoubleRow matmuls (contraction 256/chunk)
        tT8_3 = tT8[:].rearrange("p (k m) -> p k m", k=KC)
        vTg = vpool.tile([128, JC * NG], F16, name="vTg", tag="vTg")
        v4 = vTg[:].rearrange("p (j i n) -> p j i n", j=JC, i=GI)
        for jb in range(JC):
            psum = pv.tile([128, NG], F32, name="pvt", tag="pvt")
            for k2 in range(KC2):
                nc.tensor.matmul(
                    psum[:],
                    wv8_v[:, 2 * k2 : 2 * k2 + 2, jb * 128 : (jb + 1) * 128],
                    tT8_3[:, 2 * k2 : 2 * k2 + 2, :],
                    start=(k2 == 0),
                    stop=(k2 == KC2 - 1),
                    perf_mode=mybir.MatmulPerfMode.DoubleRow,
                )
            nc.vector.tensor_add(
                v4[:, jb],
                psum[:].rearrange("p (i n) -> p i n", i=GI),
                vpos3[:, jb : jb + 1, :].broadcast_to((128, GI, N)),
            )
        # v0s = v0/32 so pprod = vT' * v0s = 64 * v * v0 stays in fp16 range
        v0s = spool.tile([128, JC * GI], F16, name="v0s", tag="v0s")
        nc.scalar.activation(
            v0s[:].rearrange("p (j i) -> p j i", j=JC),
            v4[:, :, :, 0],
            mybir.ActivationFunctionType.Copy, scale=2.0 ** -16,
        )
        return vTg, v0s

    def attnS(g, vTg, v0s):
        # S' = 64*S per head via masked matmuls over pprod
        v3 = vTg[:].rearrange("p (j m) -> p j m", j=JC)
        v0s4 = v0s[:].rearrange("p (j i o) -> p j i o", j=JC, o=1)
        psum_S = pS.tile([heads, NG], F32, name="psS", tag="psS")
        for jb in range(JC):
            p = apool.tile([128, NG], F16, name="pprod", tag="pprod")
            nc.vector.tensor_mul(
                p[:].rearrange("p (i n) -> p i n", i=GI),
                v3[:, jb].rearrange("p (i n) -> p i n", i=GI),
                v0s4[:, jb].broadcast_to((128, GI, N)),
            )
            nc.tensor.matmul(
                psum_S[:],
                maskT_sb[:, jb * heads : (jb + 1) * heads],
                p[:],
                start=(jb == 0),
                stop=(jb == JC - 1),
            )
        return psum_S

    def attnAV(g, vTg, psum_S):
        # A = softmax(S); u' = SV * sum_m A[h(c), m] v[c, m] -> uT fp8
        v3 = vTg[:].rearrange("p (j m) -> p j m", j=JC)
        e_sb = apool.tile([heads, NG], F32, name="esb", tag="esb")
        nc.scalar.activation(
            e_sb[:], psum_S[:], mybir.ActivationFunctionType.Exp,
            scale=scale_exp,
        )
        d_sb = apool.tile([heads, GI], F32, name="dsb", tag="dsb")
        nc.vector.reduce_sum(
            d_sb[:],
            e_sb[:].rearrange("p (i n) -> p i n", i=GI),
            axis=mybir.AxisListType.X,
        )
        r_sb = apool.tile([heads, GI], F32, name="rsb", tag="rsb")
        nc.vector.reciprocal(r_sb[:], d_sb[:])
        a_sb = apool.tile([heads, NG], F16, name="asb", tag="asb")
        nc.vector.tensor_mul(
            a_sb[:].rearrange("p (i n) -> p i n", i=GI),
            e_sb[:].rearrange("p (i n) -> p i n", i=GI),
            r_sb[:].rearrange("p (i o) -> p i o", o=1).broadcast_to((heads, GI, N)),
        )
        for jb in range(JC):
            psum_a = pA.tile([128, NG], F32, name="psA", tag="psA")
            nc.tensor.matmul(
                psum_a[:],
                mask2_sb[:, jb * 128 : (jb + 1) * 128],
                a_sb[:],
                start=True,
                stop=True,
            )
            p2 = apool.tile([128, NG], F32, name="p2", tag="p2")
            nc.vector.tensor_mul(p2[:], psum_a[:], v3[:, jb])
            ctx8 = apool.tile([128, GI], F32, name="ctx8", tag="ctx8")
            nc.vector.reduce_sum(
                ctx8[:],
                p2[:].rearrange("p (i n) -> p i n", i=GI),
                axis=mybir.AxisListType.X,
            )
            nc.scalar.activation(
                uT_v[:, jb, g * GI : (g + 1) * GI], ctx8[:],
                mybir.ActivationFunctionType.Copy, scale=SU / SV,
            )

    def xpath(ti):
        # out rows for spatial tokens: x @ W2.T + pconst (one-hot matmul)
        m0 = ti * 128
        mw = min(128, XTOK - m0)
        osb = opool.tile([128, OUT], F32, name="osb", tag="osb")
        for oc in range(OC2):
            psum = po.tile([128, 512], F32, name="pso", tag="pso")
            for kc in range(KC):
                nc.tensor.matmul(
                    psum[:mw, :],
                    xT_v[:, kc, m0 : m0 + mw],
                    w2_sb[kc][:, oc * 512 : (oc + 1) * 512],
                    start=(kc == 0),
                    stop=False,
                )
            nc.tensor.matmul(
                psum[:mw, :],
                oneh_sb[:, m0 : m0 + mw],
                pcm_sb[:, oc * 512 : (oc + 1) * 512],
                start=False,
                stop=True,
            )
            nc.vector.tensor_copy(
                osb[:mw, oc * 512 : (oc + 1) * 512], psum[:mw, :]
            )
        # DMA out, splitting runs at item boundaries (row = j + j//49 + 1
        # skips each item's token-0 row)
        j = m0
        while j < m0 + mw:
            i = j // HW
            je = min((i + 1) * HW, m0 + mw)
            r0 = j + i + 1
            nc.sync.dma_start(
                out_flat[r0 : r0 + (je - j), :], osb[j - m0 : je - m0, :]
            )
            j = je

    def out0proj():
        # out0 = u @ Wc.T: psum = (SU*ctx)@(SW*Wc) -> scale 1/(SU*SW)
        o0 = opool.tile([IPC, OUT], F32, name="o0sb", tag="o0sb")
        for oc in range(OC2):
            psum = po.tile([128, 512], F32, name="ps0", tag="pso")
            for kc in range(KC):
                nc.tensor.matmul(
                    psum[:IPC, :],
                    uT_v[:, kc, :],
                    wc8_sb[kc][:, oc * 512 : (oc + 1) * 512],
                    start=(kc == 0),
                    stop=(kc == KC - 1),
                )
            nc.scalar.activation(
                o0[:, oc * 512 : (oc + 1) * 512], psum[:IPC, :],
                mybir.ActivationFunctionType.Copy, scale=1.0 / (SU * SW),
            )
        nc.sync.dma_start(out_d.ap()[:, 0, :], o0[:])

    # ---- schedule: fp8 vproj / attention pipelined with fp16 x-path tiles
    if variant == "full":
        vT0, v0s0 = vproj(0, build_tT8(0))
        vT1, v0s1 = vproj(1, build_tT8(1))
        s0 = attnS(0, vT0, v0s0)
        xpath(0); xpath(1)
        attnAV(0, vT0, s0)
        xpath(2)
        vT2, v0s2 = vproj(2, build_tT8(2))
        s1 = attnS(1, vT1, v0s1)
        xpath(3); xpath(4)
        attnAV(1, vT1, s1)
        xpath(5)
        vT3, v0s3 = vproj(3, build_tT8(3))
        s2 = attnS(2, vT2, v0s2)
        xpath(6); xpath(7)
        attnAV(2, vT2, s2)
        xpath(8)
        s3 = attnS(3, vT3, v0s3)
        xpath(9); xpath(10)
        attnAV(3, vT3, s3)
        xpath(11); xpath(12)
        out0proj()
    elif variant == "vproj":
        for g in range(G):
            vproj(g, build_tT8(g))
    elif variant == "xpath":
        for it in range(IPC):
            nc.sync.dma_start(
                xT_v[:, :, it * HW : (it + 1) * HW],
                x_d.ap()[it].rearrange("(k p) n -> p k n", p=128),
            )
        for ti in range(NT):
            xpath(ti)
    elif variant == "attn":
        for g in range(G):
            vTg, v0s = vproj(g, build_tT8(g))
            attnAV(g, vTg, attnS(g, vTg, v0s))
        out0proj()


_NC_CACHE = {}
_RUN_CACHE = {}


def _get_nc(heads):
    if heads not in _NC_CACHE:
        _NC_CACHE[heads] = build_kernel(heads=heads)
    return _NC_CACHE[heads]


def _run(nc, in_maps):
    """run_bass_kernel_spmd equivalent (axon/PJRT path) with: the jitted
    executable cached across calls, weight-like inputs passed replicated
    (uploaded once, not 8x), and donated output buffers created on device
    (no zero upload)."""
    import jax
    import jax.numpy as jnp
    import numpy as _np
    from jax.sharding import Mesh, PartitionSpec, NamedSharding
    from jax.experimental.shard_map import shard_map
    import concourse.mybir as mb
    from concourse import bass2jax as b2j

    # inputs where every core got the identical array object -> replicated
    replicated = {
        nm
        for nm in in_maps[0]
        if all(m[nm] is in_maps[0][nm] for m in in_maps)
    }

    key = id(nc)
    if key not in _RUN_CACHE:
        b2j.install_neuronx_cc_hook()
        in_names, out_names, out_avals = [], [], []
        partition_name = (
            nc.partition_id_tensor.name if nc.partition_id_tensor else None
        )
        for alloc in nc.m.functions[0].allocations:
            if not isinstance(alloc, mb.MemoryLocationSet):
                continue
            name = alloc.memorylocations[0].name
            if alloc.kind == "ExternalInput":
                if name != partition_name:
                    in_names.append(name)
            elif alloc.kind == "ExternalOutput":
                shape = tuple(alloc.tensor_shape)
                dtype = mb.dt.np(alloc.dtype)
                out_names.append(name)
                out_avals.append(jax.core.ShapedArray(shape, dtype))
        n_params = len(in_names)
        n_outs = len(out_avals)
        all_names = list(in_names) + list(out_names)
        if partition_name is not None:
            all_names.append(partition_name)
        donate = tuple(range(n_params, n_params + n_outs))

        def _body(*args):
            operands = list(args)
            if partition_name is not None:
                operands.append(b2j.partition_id_tensor())
            outs = b2j._bass_exec_p.bind(
                *operands,
                out_avals=tuple(out_avals),
                in_names=tuple(all_names),
                out_names=tuple(out_names),
                lowering_input_output_aliases=(),
                sim_require_finite=True,
                sim_require_nnan=True,
                nc=nc,
            )
            return tuple(outs)

        devices = jax.devices()[:CORES]
        mesh = Mesh(_np.asarray(devices), ("core",))
        in_specs = tuple(
            PartitionSpec() if nm in replicated else PartitionSpec("core")
            for nm in in_names
        ) + (PartitionSpec("core"),) * n_outs
        out_specs = (PartitionSpec("core"),) * n_outs
        sharded = jax.jit(
            shard_map(
                _body, mesh=mesh, in_specs=in_specs, out_specs=out_specs,
                check_rep=False,
            ),
            donate_argnums=donate,
            keep_unused=True,
        )
        zeros_fns = [
            jax.jit(
                (lambda shape, dtype: lambda: jnp.zeros(shape, dtype))(
                    (CORES * av.shape[0], *av.shape[1:]), av.dtype
                ),
                out_shardings=NamedSharding(mesh, PartitionSpec("core")),
            )
            for av in out_avals
        ]
        _RUN_CACHE[key] = (
            sharded, in_names, out_names, out_avals, zeros_fns, replicated
        )

    sharded, in_names, out_names, out_avals, zeros_fns, replicated_c = (
        _RUN_CACHE[key]
    )
    assert replicated == replicated_c, "replication pattern changed"
    args = [
        _np.asarray(in_maps[0][nm])
        if nm in replicated
        else _np.concatenate([_np.asarray(m[nm]) for m in in_maps], axis=0)
        for nm in in_names
    ]
    dev_zeros = [f() for f in zeros_fns]
    out_arrs = sharded(*args, *dev_zeros)
    return [
        {
            nm: _np.asarray(out_arrs[i]).reshape(CORES, *out_avals[i].shape)[c]
            for i, nm in enumerate(out_names)
        }
        for c in range(CORES)
    ]


# ---------------------------------------------------------------- host side
def _fp8(a):
    f8np = mybir.dt.np(F8)  # ml_dtypes.float8_e4m3 (TRN range, max 240)
    return np.clip(a, -240.0, 240.0).astype(f8np)


def make_in_maps(inputs, heads=H):
    x = np.asarray(inputs["x"], np.float32)
    pos_emb = np.asarray(inputs["pos_emb"], np.float32)
    Wv = np.asarray(inputs["Wv"], np.float32)
    bv = np.asarray(inputs["bv"], np.float32)
    Wc = np.asarray(inputs["Wc"], np.float32)
    bc = np.asarray(inputs["bc"], np.float32)
    num_heads = int(np.asarray(inputs["num_heads"]))
    assert num_heads == heads and x.shape == (B, C, S, S)
    assert 1 <= heads <= 128 and C % heads == 0

    wv8 = _fp8(64.0 * Wv.T)                       # [C(k), C(c)]
    W2 = Wc @ Wv                                  # [OUT, C]
    w2T = np.ascontiguousarray(W2.T).astype(np.float16)   # [C, OUT]
    wc8 = _fp8(SW * Wc.T)                         # [C, OUT]

    # vposT[128, kc*50 + n] = SV * (pos_emb @ Wv.T + bv).T chunk-tiled
    vpos = SV * (pos_emb @ Wv.T + bv).astype(np.float32)  # [N, C]
    vposT = np.empty((128, KC * N), np.float32)
    for kc in range(KC):
        vposT[:, kc * N : (kc + 1) * N] = vpos[:, kc * 128 : (kc + 1) * 128].T

    # maskT[p, kc*heads + h] = 1 if channel kc*128+p belongs to head h
    head_of = np.arange(C) // (C // heads)
    maskT = np.zeros((128, KC * heads), np.float16)
    mask2 = np.zeros((heads, KC * 128), np.float16)
    for kc in range(KC):
        for p in range(128):
            h = head_of[kc * 128 + p]
            maskT[p, kc * heads + h] = 1.0
            mask2[h, kc * 128 + p] = 1.0

    # x-path pos constant: out[n>=1] += pconst[n] via one-hot matmul
    # oneh[p, j] = 1 iff p == j % 49; pcm[p] = pconst[p+1]
    oneh = np.zeros((128, NT * 128), np.float16)
    j = np.arange(XTOK)
    oneh[j % HW, j] = 1.0
    pconst = pos_emb @ W2.T + bv @ Wc.T           # [N, OUT]
    pcm = np.zeros((128, OUT), np.float16)
    pcm[:HW] = pconst[1:].astype(np.float16)

    xr16 = np.ascontiguousarray(x.reshape(B, C, HW).astype(np.float16))
    in_maps = []
    for core in range(CORES):
        in_maps.append(
            {
                "x": xr16[core * IPC : (core + 1) * IPC],
                "wv8": wv8,
                "w2T": w2T,
                "wc8": wc8,
                "vposT": vposT,
                "maskT": maskT,
                "mask2": mask2,
                "oneh": oneh,
                "pcm": pcm,
            }
        )

    return in_maps


def kernel(**inputs):
    from concourse._compat import axon_active

    heads = int(np.asarray(inputs["num_heads"]))
    in_maps = make_in_maps(inputs, heads)
    nc = _get_nc(heads)
    if axon_active():
        results = _run(nc, in_maps)
    else:
        results = run_bass_kernel_spmd(nc, in_maps, list(range(CORES))).results
    out = np.concatenate([results[i]["out"] for i in range(CORES)], axis=0)
    out = np.ascontiguousarray(out, dtype=np.float32)
    bc = np.asarray(inputs["bc"], np.float32)
    if bc.any():
        out = out + bc[None, None, :]
    return out


# revision 33
# speedup vs baseline: 1.7414x; 1.3466x over previous
"""AttentionPool2d Trainium2 kernel (8-core data parallel over batch).

Math (per batch item), exploiting that only query token 0 survives into the
output: tokens t = [mean(x); x_tokens] + pos_emb; v = t @ Wv.T + bv;
out[1:] = v[1:] @ Wc.T + bc; out[0] = softmax(q0.K/sqrt(hd)) V @ Wc.T + bc
with q0 = K = V = v (per head).

Split into two precision domains:
 - tokens 1..49 (98% of the output mass) bypass v entirely:
   out[n] = x_n @ W2.T + pconst[n], W2 = Wc @ Wv (host-precomputed, fp16
   matmul on device). pconst[n] = pos_n @ W2.T + bv @ Wc.T is folded into
   the same PSUM accumulation via a one-hot 17th matmul.
 - token 0 goes through attention, where ~4% relative error is invisible
   in the full-output l2 (weight ~1/50): v is computed with fp8-e4m3
   DoubleRow matmuls (2x PE throughput; scales 32*t and 64*Wv keep
   everything in e4m3 normal range, TRN max 240), attention runs on
   vT' = 2048*v fp16, and out0 = u @ Wc with u,Wc in fp8.

Measured end-to-end l2 vs reference ~8e-4 (budget 2e-2).
"""

import numpy as np

import bass_rust
import concourse.bass as bass
import concourse.mybir as mybir
import concourse.tile as tile
from concourse.bass_utils import run_bass_kernel_spmd
from concourse.tile_scheduler import PROC_NAME_TO_IDX
from contextlib import ExitStack

# ---------------------------------------------------------------- constants
B, C, S = 256, 2048, 7
HW = S * S              # 49 spatial tokens
N = HW + 1              # 50 tokens incl. mean token
H, OUT = 32, 1024       # default num_heads; build is parameterized
HD = C // H
CORES = 8
IPC = B // CORES        # 32 items per core
GI = 8                  # items per group
G = IPC // GI           # 4 groups
NG = GI * N             # 400 moving columns per group
KC = C // 128           # 16 contraction chunks
KC2 = KC // 2           # 8 fp8 DoubleRow super-chunks
JC = C // 128           # 16 output-channel chunks of v
XTOK = IPC * HW         # 1568 spatial tokens per core (x-path)
NT = (XTOK + 127) // 128  # 13 x-path token tiles
OC2 = OUT // 512        # 2 out-projection column chunks

# fp8 scaling: tT8 = 32*t, wv8 = 64*Wv  =>  psum = 2048*(t@Wv.T)
SV = 2048.0             # vT' = SV * v
SU = 32.0               # uT = SU * ctx
SW = 64.0               # wc8 = SW * Wc

F8 = mybir.dt.float8e4
F16 = mybir.dt.float16
F32 = mybir.dt.float32

N_PROCS = 27


# ------------------------------------------------------- tile/walrus patches
def _patched_drain_and_barrier(self, tick_clock, wait_clock):
    """Stock tail drain carries one wait per ticked proc; walrus here allows
    a single sync-wait per instruction. Funnel waits through SP nops."""
    nc = self.nc
    gc = tick_clock.global_clock
    ticks = [gc.peek_next(i) - 1 for i in range(N_PROCS)]
    live = [i for i in range(N_PROCS) if ticks[i] > 0]
    sp_clock = wait_clock.engine_clocks[PROC_NAME_TO_IDX["SP"]]
    for p in live:
        vc = bass_rust.VectorClock()
        vc.require_at_least(p, ticks[p])
        nop = nc.sync.nop(nofuse=True, hint="tail_wait_funnel")
        wait_clock.add_sem_waits(
            nop.ins, bass_rust.ScopedClock({None: vc}), cur_clock=sp_clock
        )
        sp_clock.require_at_least(None, p, ticks[p])
    drain_inst = nc.sync.drain()
    wait_clock.add_sem_waits(
        drain_inst.ins, bass_rust.ScopedClock({None: gc}), cur_clock=sp_clock
    )
    nc.all_engine_barrier()
    assert self.sems is not None
    popped = nc._tile_sem_poison_stack.pop()
    assert popped is self._sem_poison
    nc.clear_and_free_semaphores(list(self.sems.allocated().values()))
    nc.all_engine_barrier()


tile.TileContext._drain_and_barrier = _patched_drain_and_barrier


def fix_excess_waits(nc, max_waits=1):
    """Hoist excess per-instruction sync-waits onto injected same-engine
    NoOps placed immediately before the offender (engine streams run in
    basic-block order)."""
    for bb in nc.m.functions[0].blocks:
        insts = bb.instructions
        if not any(
            i.sync_info and i.sync_info.on_wait and len(i.sync_info.on_wait) > max_waits
            for i in insts
        ):
            continue
        out = []
        for inst in insts:
            si = inst.sync_info
            if si and si.on_wait and len(si.on_wait) > max_waits:
                waits = list(si.on_wait)
                extra, keep = waits[:-max_waits], waits[-max_waits:]
                for i in range(0, len(extra), max_waits):
                    chunk = extra[i : i + max_waits]
                    nop = mybir.InstNoOp(
                        name=nc.get_next_instruction_name(), ins=[], outs=[]
                    )
                    nop.engine = inst.engine
                    nop.sync_info = bass_rust.SyncInfo(on_wait=chunk, on_update=[])
                    nc.register_instruction(nop)
                    out.append(nop)
                si.on_wait = keep
            out.append(inst)
        bb.instructions = out


def dedup_ldweights(nc):
    """Drop an InstLdweights whose weights AP (and modes) match the previous
    weight load on the PE stream — the PE array keeps the stationary operand
    across matmuls, so a reload of identical weights only burns LDW cycles.
    Only loads carrying no sem waits/updates are removed."""
    import concourse.mybir as mb

    for bb in nc.m.functions[0].blocks:
        last = None
        out = []
        for inst in bb.instructions:
            if isinstance(inst, mb.InstLdweights):
                s = (
                    str(inst.ins[0]),
                    str(getattr(inst, "perf_mode", None)),
                    str(getattr(inst, "is_transpose", None)),
                    str(getattr(inst, "tile_position", None)),
                )
                clean = not inst.sync_info or (
                    not inst.sync_info.on_wait and not inst.sync_info.on_update
                )
                if s == last and clean:
                    continue
                last = s
            out.append(inst)
        bb.instructions = out


# ------------------------------------------------------------- kernel build
def build_kernel(reps=1, variant="full", heads=H, unroll=False):
    nc = bass.Bass("TRN2", target_bir_lowering=False, debug=False)

    x_d = nc.dram_tensor("x", [IPC, C, HW], F16, kind="ExternalInput")
    wv8_d = nc.dram_tensor("wv8", [C, C], F8, kind="ExternalInput")
    w2_d = nc.dram_tensor("w2T", [C, OUT], F16, kind="ExternalInput")
    wc8_d = nc.dram_tensor("wc8", [C, OUT], F8, kind="ExternalInput")
    vpos_d = nc.dram_tensor("vposT", [128, KC * N], F32, kind="ExternalInput")
    maskT_d = nc.dram_tensor("maskT", [128, KC * heads], F8, kind="ExternalInput")
    mask2_d = nc.dram_tensor("mask2", [heads, KC * 128], F16, kind="ExternalInput")
    oneh_d = nc.dram_tensor("oneh", [128, NT * 128], F16, kind="ExternalInput")
    pcm_d = nc.dram_tensor("pcm", [128, OUT], F16, kind="ExternalInput")
    # x-path rows land contiguously (token-major, no token-0 gaps); host
    # reassembles [IPC, N, OUT] from outx + out0.
    outx_d = nc.dram_tensor("outx", [XTOK, OUT], F16, kind="ExternalOutput")
    out0_d = nc.dram_tensor("out0", [IPC, OUT], F16, kind="ExternalOutput")

    with tile.TileContext(nc) as tc, ExitStack() as ctx:
        wv_pool = ctx.enter_context(tc.tile_pool(name="wv", bufs=1))
        w2_pool = ctx.enter_context(tc.tile_pool(name="w2", bufs=1))
        wc_pool = ctx.enter_context(tc.tile_pool(name="wc", bufs=1))
        cpool = ctx.enter_context(tc.tile_pool(name="consts", bufs=1))
        xpool = ctx.enter_context(tc.tile_pool(name="xT", bufs=1))
        spool = ctx.enter_context(tc.tile_pool(name="small", bufs=2))
        tpool = ctx.enter_context(tc.tile_pool(name="tT8", bufs=2))
        vpool = ctx.enter_context(tc.tile_pool(name="vT", bufs=2))
        apool = ctx.enter_context(tc.tile_pool(name="attn", bufs=2))
        ppool = ctx.enter_context(tc.tile_pool(name="pp8", bufs=1))
        opool = ctx.enter_context(tc.tile_pool(name="outsb", bufs=2))
        upool = ctx.enter_context(tc.tile_pool(name="uT", bufs=1))
        pv = ctx.enter_context(tc.tile_pool(name="pv", bufs=2, space="PSUM"))
        pS = ctx.enter_context(tc.tile_pool(name="pS", bufs=1, space="PSUM"))
        pA = ctx.enter_context(tc.tile_pool(name="pA", bufs=3, space="PSUM"))
        po = ctx.enter_context(tc.tile_pool(name="po", bufs=2, space="PSUM"))

        # ---- resident weights/constants (loaded outside the rep loop)
        wv8_sb = wv_pool.tile([128, KC * C], F8, name="wv8")
        for kc in range(KC):
            nc.sync.dma_start(
                wv8_sb[:, kc * C : (kc + 1) * C],
                wv8_d.ap()[kc * 128 : (kc + 1) * 128, :],
            )
        w2_sb, wc8_sb = [], []
        for kc in range(KC):
            w = w2_pool.tile([128, OUT], F16, name=f"w2{kc}", tag=f"w2{kc}")
            nc.sync.dma_start(w[:], w2_d.ap()[kc * 128 : (kc + 1) * 128, :])
            w2_sb.append(w)
            w8 = wc_pool.tile([128, OUT], F8, name=f"wc{kc}", tag=f"wc{kc}")
            nc.sync.dma_start(w8[:], wc8_d.ap()[kc * 128 : (kc + 1) * 128, :])
            wc8_sb.append(w8)
        vpos_sb = cpool.tile([128, KC * N], F32, name="vpos")
        nc.sync.dma_start(vpos_sb[:], vpos_d.ap())
        maskT_sb = cpool.tile([128, KC * heads], F8, name="maskT")
        nc.sync.dma_start(maskT_sb[:], maskT_d.ap())
        mask2_sb = cpool.tile([heads, KC * 128], F16, name="mask2")
        nc.sync.dma_start(mask2_sb[:], mask2_d.ap())
        oneh_sb = cpool.tile([128, NT * 128], F16, name="oneh")
        nc.sync.dma_start(oneh_sb[:], oneh_d.ap())
        pcm_sb = cpool.tile([128, OUT], F16, name="pcm")
        nc.sync.dma_start(pcm_sb[:], pcm_d.ap())

        # x tokens resident in [channel, kc-major global token] layout:
        # xT[p, kc, j] = x[item j//49, kc*128+p, j%49], fp16
        xT_sb = xpool.tile([128, KC * XTOK], F16, name="xTall")
        # uT[p, kc, i] = SU * ctx[item i, kc*128+p], fp8
        uT_sb = upool.tile([128, KC * IPC], F8, name="uT")

        def work():
            body(nc, tc, x_d, (outx_d, out0_d), wv8_sb, w2_sb, wc8_sb, vpos_sb,
                 maskT_sb, mask2_sb, oneh_sb, pcm_sb, xT_sb, uT_sb,
                 spool, tpool, vpool, apool, ppool, opool, pv, pS, pA, po,
                 variant, heads)

        if reps == 1:
            work()
        elif unroll:
            for _ in range(reps):
                work()
        else:
            with tc.For_i(0, reps, 1):
                work()

    dedup_ldweights(nc)
    fix_excess_waits(nc)
    return nc


def body(nc, tc, x_d, outs, wv8_sb, w2_sb, wc8_sb, vpos_sb, maskT_sb,
         mask2_sb, oneh_sb, pcm_sb, xT_sb, uT_sb, spool, tpool, vpool,
         apool, ppool, opool, pv, pS, pA, po, variant="full", heads=H):
    outx_d, out0_d = outs
    scale_exp = float((C // heads) ** -0.5)
    wv8_v = wv8_sb[:].rearrange("p (k c) -> p k c", k=KC)
    xT_v = xT_sb[:].rearrange("p (k j) -> p k j", k=KC)
    uT_v = uT_sb[:].rearrange("p (k i) -> p k i", k=KC)
    vpos3 = vpos_sb[:].rearrange("p (k n) -> p k n", k=KC)

    def build_tT8(g):
        # tT8 layout: [128, KC*(GI*N)] fp8 = 32*t, kc-major so the DoubleRow
        # moving operand spans 2 adjacent kc subtiles: [p, 2, 400]
        tT8 = tpool.tile([128, KC * GI * N], F8, name="tT8", tag="tT8")
        t4 = tT8[:].rearrange("p (k i n) -> p k i n", k=KC, i=GI)
        for it in range(GI):
            gi = g * GI + it
            dst = xT_v[:, :, gi * HW : (gi + 1) * HW]
            nc.sync.dma_start(
                dst, x_d.ap()[gi].rearrange("(k p) n -> p k n", p=128)
            )
            # spatial tokens: fp8(32 * x)
            nc.scalar.activation(
                t4[:, :, it, 1:N], dst,
                mybir.ActivationFunctionType.Copy, scale=32.0,
            )
            # mean token: fp8(32/49 * sum_s x)
            xsum = spool.tile([128, KC], F32, name="xsum", tag="xsum")
            nc.vector.reduce_sum(xsum[:], dst, axis=mybir.AxisListType.X)
            nc.scalar.activation(
                t4[:, :, it, 0], xsum[:],
                mybir.ActivationFunctionType.Copy, scale=32.0 / HW,
            )
        return tT8

    def vproj(g, tT8):
        # vT' = SV * v fp16 via fp8 DoubleRow matmuls (contraction 256/chunk).
        # Per jb, also emit the attention prework on DVE/Act so attnS's
        # matmuls have no cross-engine wait: v0s slice + fp8 pprod = v * v0.
        tT8_3 = tT8[:].rearrange("p (k m) -> p k m", k=KC)
        vTg = vpool.tile([128, JC * NG], F16, name="vTg", tag="vTg")
        v4 = vTg[:].rearrange("p (j i n) -> p j i n", j=JC, i=GI)
        v0s = spool.tile([128, JC * GI], F16, name="v0s", tag="v0s")
        v0s4 = v0s[:].rearrange("p (j i o) -> p j i o", j=JC, o=1)
        pp8 = ppool.tile([128, KC * NG], F8, name="pp8", tag="pp8")
        pp8_4 = pp8[:].rearrange("p (j i n) -> p j i n", j=JC, i=GI)
        for jb in range(JC):
            psum = pv.tile([128, NG], F32, name="pvt", tag="pvt")
            for k2 in range(KC2):
                nc.tensor.matmul(
                    psum[:],
                    wv8_v[:, 2 * k2 : 2 * k2 + 2, jb * 128 : (jb + 1) * 128],
                    tT8_3[:, 2 * k2 : 2 * k2 + 2, :],
                    start=(k2 == 0),
                    stop=(k2 == KC2 - 1),
                    perf_mode=mybir.MatmulPerfMode.DoubleRow,
                )
            nc.vector.tensor_add(
                v4[:, jb],
                psum[:].rearrange("p (i n) -> p i n", i=GI),
                vpos3[:, jb : jb + 1, :].broadcast_to((128, GI, N)),
            )
            # v0s = v0/32 (fp16-normal range)
            nc.scalar.activation(
                v0s4[:, jb, :, 0], v4[:, jb, :, 0],
                mybir.ActivationFunctionType.Copy, scale=2.0 ** -16,
            )
            # pprod = (vT'/64) * (v0/32) = v * v0, fp8
            nc.vector.scalar_tensor_tensor(
                pp8_4[:, jb],
                v4[:, jb],
                2.0 ** -6,
                v0s4[:, jb].broadcast_to((128, GI, N)),
                op0=mybir.AluOpType.mult,
                op1=mybir.AluOpType.mult,
            )
        return vTg, pp8

    def attnS(g, vTg, pp8):
        # S per head: fp8 DoubleRow masked matmuls over pprod
        maskT_v = maskT_sb[:].rearrange("p (k h) -> p k h", k=KC)
        pp8_3 = pp8[:].rearrange("p (k m) -> p k m", k=KC)
        psum_S = pS.tile([heads, NG], F32, name="psS", tag="psS")
        for k2 in range(KC2):
            nc.tensor.matmul(
                psum_S[:],
                maskT_v[:, 2 * k2 : 2 * k2 + 2, :],
                pp8_3[:, 2 * k2 : 2 * k2 + 2, :],
                start=(k2 == 0),
                stop=(k2 == KC2 - 1),
                perf_mode=mybir.MatmulPerfMode.DoubleRow,
            )
        return psum_S

    def xpath_tile(ti):
        # one token tile, both out-column halves: kc-outer so each stationary
        # x-token chunk is loaded once and streams both w2 halves; Act-engine
        # PSUM->SBUF copies, one whole-tile DMA
        m0 = ti * 128
        mw = min(128, XTOK - m0)
        ps = [po.tile([128, 512], F32, name=f"pso{oc}", tag="pso")
              for oc in range(OC2)]
        for kc in range(KC):
            for oc in range(OC2):
                nc.tensor.matmul(
                    ps[oc][:mw, :],
                    xT_v[:, kc, m0 : m0 + mw],
                    w2_sb[kc][:, oc * 512 : (oc + 1) * 512],
                    start=(kc == 0),
                    stop=False,
                    skip_group_check=True,
                )
        for oc in range(OC2):
            nc.tensor.matmul(
                ps[oc][:mw, :],
                oneh_sb[:, m0 : m0 + mw],
                pcm_sb[:, oc * 512 : (oc + 1) * 512],
                start=False,
                stop=True,
                skip_group_check=True,
            )
        osb = opool.tile([128, OUT], F16, name="osb", tag="osb")
        for oc in range(OC2):
            nc.scalar.activation(
                osb[:mw, oc * 512 : (oc + 1) * 512], ps[oc][:mw, :],
                mybir.ActivationFunctionType.Copy,
            )
        nc.sync.dma_start(outx_d.ap()[m0 : m0 + mw, :], osb[:mw, :])

    def attnAV(g, vTg, psum_S, fillers=()):
        # A = softmax(S); u' = SV * sum_m A[h(c), m] v[c, m] -> uT fp8.
        # fillers: x-path oc-blocks interleaved so the PE streams matmuls
        # while the DVE drains each psA bank.
        v3 = vTg[:].rearrange("p (j m) -> p j m", j=JC)
        e_sb = apool.tile([heads, NG], F32, name="esb", tag="esb")
        nc.scalar.activation(
            e_sb[:], psum_S[:], mybir.ActivationFunctionType.Exp,
            scale=scale_exp,
        )
        d_sb = apool.tile([heads, GI], F32, name="dsb", tag="dsb")
        nc.vector.reduce_sum(
            d_sb[:],
            e_sb[:].rearrange("p (i n) -> p i n", i=GI),
            axis=mybir.AxisListType.X,
        )
        r_sb = apool.tile([heads, GI], F32, name="rsb", tag="rsb")
        nc.vector.reciprocal(r_sb[:], d_sb[:])
        a_sb = apool.tile([heads, NG], F16, name="asb", tag="asb")
        nc.vector.tensor_mul(
            a_sb[:].rearrange("p (i n) -> p i n", i=GI),
            e_sb[:].rearrange("p (i n) -> p i n", i=GI),
            r_sb[:].rearrange("p (i o) -> p i o", o=1).broadcast_to((heads, GI, N)),
        )
        fillers = list(fillers)
        chunks = [range(0, 3), range(3, 6), range(6, 9), range(9, 12),
                  range(12, 16)]
        for ci, chunk in enumerate(chunks):
            for jb in chunk:
                psum_a = pA.tile([128, NG], F32, name="psA", tag="psA")
                nc.tensor.matmul(
                    psum_a[:],
                    mask2_sb[:, jb * 128 : (jb + 1) * 128],
                    a_sb[:],
                    start=True,
                    stop=True,
                )
                p2 = apool.tile([128, NG], F16, name="p2", tag="p2")
                nc.vector.tensor_mul(p2[:], psum_a[:], v3[:, jb])
                ctx8 = apool.tile([128, GI], F32, name="ctx8", tag="ctx8")
                nc.vector.reduce_sum(
                    ctx8[:],
                    p2[:].rearrange("p (i n) -> p i n", i=GI),
                    axis=mybir.AxisListType.X,
                )
                nc.scalar.activation(
                    uT_v[:, jb, g * GI : (g + 1) * GI], ctx8[:],
                    mybir.ActivationFunctionType.Copy, scale=SU / SV,
                )
            if ci < len(fillers):
                xpath_tile(fillers[ci])
        for fi in range(len(chunks), len(fillers)):
            xpath_tile(fillers[fi])

    def out0proj():
        # out0 = u @ Wc.T: psum = (SU*ctx)@(SW*Wc) -> scale 1/(SU*SW)
        o0 = opool.tile([IPC, OUT], F16, name="o0sb", tag="o0sb")
        for oc in range(OC2):
            psum = po.tile([128, 512], F32, name="ps0", tag="pso")
            for kc in range(KC):
                nc.tensor.matmul(
                    psum[:IPC, :],
                    uT_v[:, kc, :],
                    wc8_sb[kc][:, oc * 512 : (oc + 1) * 512],
                    start=(kc == 0),
                    stop=(kc == KC - 1),
                )
            nc.scalar.activation(
                o0[:, oc * 512 : (oc + 1) * 512], psum[:IPC, :],
                mybir.ActivationFunctionType.Copy, scale=1.0 / (SU * SW),
            )
        nc.sync.dma_start(out0_d.ap()[:], o0[:])

    # ---- schedule: fp8 vproj / attention pipelined with fp16 x-path tiles
    if variant == "full":
        vt0 = vproj(0, build_tT8(0))
        s_prev = attnS(0, *vt0)
        v_prev = vt0[0]
        for g in range(1, G + 1):
            if g < G:
                vtg = vproj(g, build_tT8(g))
            fillers = [3 * (g - 1) + t for t in range(3)]
            if g == G:
                fillers.append(12)
            attnAV(g - 1, v_prev, s_prev, fillers)
            if g < G:
                s_prev = attnS(g, *vtg)
                v_prev = vtg[0]
        out0proj()
    elif variant == "vproj":
        for g in range(G):
            vproj(g, build_tT8(g))
    elif variant in ("xpath", "xmm", "xnodma"):
        for it in range(IPC):
            nc.sync.dma_start(
                xT_v[:, :, it * HW : (it + 1) * HW],
                x_d.ap()[it].rearrange("(k p) n -> p k n", p=128),
            )
        if variant == "xmm":
            # sim bisect: matmuls only, no psum drain
            for ti in range(NT):
                for oc in range(OC2):
                    m0 = ti * 128
                    mw = min(128, XTOK - m0)
                    psum = po.tile([128, 512], F32, name="pso", tag="pso")
                    for kc in range(KC):
                        nc.tensor.matmul(
                            psum[:mw, :],
                            xT_v[:, kc, m0 : m0 + mw],
                            w2_sb[kc][:, oc * 512 : (oc + 1) * 512],
                            start=(kc == 0),
                            stop=(kc == KC - 1),
                        )
        elif variant == "xnodma":
            # sim bisect: matmuls + copies, no out DMA
            for ti in range(NT):
                for oc in range(OC2):
                    m0 = ti * 128
                    mw = min(128, XTOK - m0)
                    psum = po.tile([128, 512], F32, name="pso", tag="pso")
                    for kc in range(KC):
                        nc.tensor.matmul(
                            psum[:mw, :],
                            xT_v[:, kc, m0 : m0 + mw],
                            w2_sb[kc][:, oc * 512 : (oc + 1) * 512],
                            start=(kc == 0),
                            stop=(kc == KC - 1),
                        )
                    osb = opool.tile([128, OUT], F16, name="osb", tag="osb")
                    nc.scalar.activation(
                        osb[:mw, oc * 512 : (oc + 1) * 512], psum[:mw, :],
                        mybir.ActivationFunctionType.Copy,
                    )
        else:
            for ti in range(NT):
                xpath_tile(ti)
    elif variant == "attn":
        for g in range(G):
            vTg, v0s = vproj(g, build_tT8(g))
            attnAV(g, vTg, attnS(g, vTg, v0s))
        out0proj()


_NC_CACHE = {}
_RUN_CACHE = {}


def _get_nc(heads):
    if heads not in _NC_CACHE:
        _NC_CACHE[heads] = build_kernel(heads=heads)
    return _NC_CACHE[heads]


def _run(nc, in_maps):
    """run_bass_kernel_spmd equivalent (axon/PJRT path) with: the jitted
    executable cached across calls, weight-like inputs passed replicated
    (uploaded once, not 8x), and donated output buffers created on device
    (no zero upload)."""
    import jax
    import jax.numpy as jnp
    import numpy as _np
    from jax.sharding import Mesh, PartitionSpec, NamedSharding
    from jax.experimental.shard_map import shard_map
    import concourse.mybir as mb
    from concourse import bass2jax as b2j

    # inputs where every core got the identical array object -> replicated
    replicated = {
        nm
        for nm in in_maps[0]
        if all(m[nm] is in_maps[0][nm] for m in in_maps)
    }

    key = id(nc)
    if key not in _RUN_CACHE:
        b2j.install_neuronx_cc_hook()
        in_names, out_names, out_avals = [], [], []
        partition_name = (
            nc.partition_id_tensor.name if nc.partition_id_tensor else None
        )
        for alloc in nc.m.functions[0].allocations:
            if not isinstance(alloc, mb.MemoryLocationSet):
                continue
            name = alloc.memorylocations[0].name
            if alloc.kind == "ExternalInput":
                if name != partition_name:
                    in_names.append(name)
            elif alloc.kind == "ExternalOutput":
                shape = tuple(alloc.tensor_shape)
                dtype = mb.dt.np(alloc.dtype)
                out_names.append(name)
                out_avals.append(jax.core.ShapedArray(shape, dtype))
        n_params = len(in_names)
        n_outs = len(out_avals)
        all_names = list(in_names) + list(out_names)
        if partition_name is not None:
            all_names.append(partition_name)
        donate = tuple(range(n_params, n_params + n_outs))

        def _body(*args):
            operands = list(args)
            if partition_name is not None:
                operands.append(b2j.partition_id_tensor())
            outs = b2j._bass_exec_p.bind(
                *operands,
                out_avals=tuple(out_avals),
                in_names=tuple(all_names),
                out_names=tuple(out_names),
                lowering_input_output_aliases=(),
                sim_require_finite=True,
                sim_require_nnan=True,
                nc=nc,
            )
            return tuple(outs)

        devices = jax.devices()[:CORES]
        mesh = Mesh(_np.asarray(devices), ("core",))
        in_specs = tuple(
            PartitionSpec() if nm in replicated else PartitionSpec("core")
            for nm in in_names
        ) + (PartitionSpec("core"),) * n_outs
        out_specs = (PartitionSpec("core"),) * n_outs
        sharded = jax.jit(
            shard_map(
                _body, mesh=mesh, in_specs=in_specs, out_specs=out_specs,
                check_rep=False,
            ),
            donate_argnums=donate,
            keep_unused=True,
        )
        zeros_fns = [
            jax.jit(
                (lambda shape, dtype: lambda: jnp.zeros(shape, dtype))(
                    (CORES * av.shape[0], *av.shape[1:]), av.dtype
                ),
                out_shardings=NamedSharding(mesh, PartitionSpec("core")),
            )
            for av in out_avals
        ]
        _RUN_CACHE[key] = (
            sharded, in_names, out_names, out_avals, zeros_fns, replicated
        )

    sharded, in_names, out_names, out_avals, zeros_fns, replicated_c = (
        _RUN_CACHE[key]
    )
    assert replicated == replicated_c, "replication pattern changed"
    args = [
        _np.asarray(in_maps[0][nm])
        if nm in replicated
        else _np.concatenate([_np.asarray(m[nm]) for m in in_maps], axis=0)
        for nm in in_names
    ]
    dev_zeros = [f() for f in zeros_fns]
    out_arrs = sharded(*args, *dev_zeros)
    return [
        {
            nm: _np.asarray(out_arrs[i]).reshape(CORES, *out_avals[i].shape)[c]
            for i, nm in enumerate(out_names)
        }
        for c in range(CORES)
    ]


# ---------------------------------------------------------------- host side
def _fp8(a):
    f8np = mybir.dt.np(F8)  # ml_dtypes.float8_e4m3 (TRN range, max 240)
    return np.clip(a, -240.0, 240.0).astype(f8np)


def make_in_maps(inputs, heads=H):
    x = np.asarray(inputs["x"], np.float32)
    pos_emb = np.asarray(inputs["pos_emb"], np.float32)
    Wv = np.asarray(inputs["Wv"], np.float32)
    bv = np.asarray(inputs["bv"], np.float32)
    Wc = np.asarray(inputs["Wc"], np.float32)
    bc = np.asarray(inputs["bc"], np.float32)
    num_heads = int(np.asarray(inputs["num_heads"]))
    assert num_heads == heads and x.shape == (B, C, S, S)
    assert 1 <= heads <= 128 and C % heads == 0

    wv8 = _fp8(64.0 * Wv.T)                       # [C(k), C(c)]
    W2 = Wc @ Wv                                  # [OUT, C]
    w2T = np.ascontiguousarray(W2.T).astype(np.float16)   # [C, OUT]
    wc8 = _fp8(SW * Wc.T)                         # [C, OUT]

    # vposT[128, kc*50 + n] = SV * (pos_emb @ Wv.T + bv).T chunk-tiled
    vpos = SV * (pos_emb @ Wv.T + bv).astype(np.float32)  # [N, C]
    vposT = np.empty((128, KC * N), np.float32)
    for kc in range(KC):
        vposT[:, kc * N : (kc + 1) * N] = vpos[:, kc * 128 : (kc + 1) * 128].T

    # maskT[p, kc*heads + h] = 1 if channel kc*128+p belongs to head h
    head_of = np.arange(C) // (C // heads)
    maskT = np.zeros((128, KC * heads), mybir.dt.np(F8))
    mask2 = np.zeros((heads, KC * 128), np.float16)
    for kc in range(KC):
        for p in range(128):
            h = head_of[kc * 128 + p]
            maskT[p, kc * heads + h] = 1.0
            mask2[h, kc * 128 + p] = 1.0

    # x-path pos constant: out[n>=1] += pconst[n] via one-hot matmul
    # oneh[p, j] = 1 iff p == j % 49; pcm[p] = pconst[p+1]
    oneh = np.zeros((128, NT * 128), np.float16)
    j = np.arange(XTOK)
    oneh[j % HW, j] = 1.0
    pconst = pos_emb @ W2.T + bv @ Wc.T           # [N, OUT]
    pcm = np.zeros((128, OUT), np.float16)
    pcm[:HW] = pconst[1:].astype(np.float16)

    xr16 = np.ascontiguousarray(x.reshape(B, C, HW).astype(np.float16))
    in_maps = []
    for core in range(CORES):
        in_maps.append(
            {
                "x": xr16[core * IPC : (core + 1) * IPC],
                "wv8": wv8,
                "w2T": w2T,
                "wc8": wc8,
                "vposT": vposT,
                "maskT": maskT,
                "mask2": mask2,
                "oneh": oneh,
                "pcm": pcm,
            }
        )

    return in_maps


def kernel(**inputs):
    from concourse._compat import axon_active

    heads = int(np.asarray(inputs["num_heads"]))
    in_maps = make_in_maps(inputs, heads)
    nc = _get_nc(heads)
    if axon_active():
        results = _run(nc, in_maps)
    else:
        results = run_bass_kernel_spmd(nc, in_maps, list(range(CORES))).results
    out = np.empty((B, N, OUT), np.float32)
    for i in range(CORES):
        blk = out[i * IPC : (i + 1) * IPC]
        blk[:, 1:] = np.asarray(results[i]["outx"]).reshape(IPC, HW, OUT)
        blk[:, 0] = np.asarray(results[i]["out0"])
    bc = np.asarray(inputs["bc"], np.float32)
    if bc.any():
        out = out + bc[None, None, :]
    return out


# revision 37
# speedup vs baseline: 1.8342x; 1.0533x over previous
"""AttentionPool2d Trainium2 kernel (8-core data parallel over batch).

Math (per batch item), exploiting that only query token 0 survives into the
output: tokens t = [mean(x); x_tokens] + pos_emb; v = t @ Wv.T + bv;
out[1:] = v[1:] @ Wc.T + bc; out[0] = softmax(q0.K/sqrt(hd)) V @ Wc.T + bc
with q0 = K = V = v (per head).

Split into two precision domains:
 - tokens 1..49 (98% of the output mass) bypass v entirely:
   out[n] = x_n @ W2.T + pconst[n], W2 = Wc @ Wv (host-precomputed, fp16
   matmul on device). pconst[n] = pos_n @ W2.T + bv @ Wc.T is folded into
   the same PSUM accumulation via a one-hot 17th matmul.
 - token 0 goes through attention, where ~4% relative error is invisible
   in the full-output l2 (weight ~1/50): v is computed with fp8-e4m3
   DoubleRow matmuls (2x PE throughput; scales 32*t and 64*Wv keep
   everything in e4m3 normal range, TRN max 240), attention runs on
   vT' = 2048*v fp16, and out0 = u @ Wc with u,Wc in fp8.

Measured end-to-end l2 vs reference ~8e-4 (budget 2e-2).
"""

import numpy as np

import bass_rust
import concourse.bass as bass
import concourse.mybir as mybir
import concourse.tile as tile
from concourse.bass_utils import run_bass_kernel_spmd
from concourse.tile_scheduler import PROC_NAME_TO_IDX
from contextlib import ExitStack

# ---------------------------------------------------------------- constants
B, C, S = 256, 2048, 7
HW = S * S              # 49 spatial tokens
N = HW + 1              # 50 tokens incl. mean token
H, OUT = 32, 1024       # default num_heads; build is parameterized
HD = C // H
CORES = 8
IPC = B // CORES        # 32 items per core
GI = 8                  # items per group
G = IPC // GI           # 4 groups
NG = GI * N             # 400 moving columns per group
KC = C // 128           # 16 contraction chunks
KC2 = KC // 2           # 8 fp8 DoubleRow super-chunks
JC = C // 128           # 16 output-channel chunks of v
XTOK = IPC * HW         # 1568 spatial tokens per core (x-path)
NT = (XTOK + 127) // 128  # 13 x-path token tiles
OC2 = OUT // 512        # 2 out-projection column chunks

# fp8 scaling: tT8 = 32*t, wv8 = 64*Wv  =>  psum = 2048*(t@Wv.T)
SV = 2048.0             # vT' = SV * v
SU = 32.0               # uT = SU * ctx
SW = 64.0               # wc8 = SW * Wc

F8 = mybir.dt.float8e4
F16 = mybir.dt.float16
F32 = mybir.dt.float32

N_PROCS = 27


# ------------------------------------------------------- tile/walrus patches
def _patched_drain_and_barrier(self, tick_clock, wait_clock):
    """Stock tail drain carries one wait per ticked proc; walrus here allows
    a single sync-wait per instruction. Funnel waits through SP nops."""
    nc = self.nc
    gc = tick_clock.global_clock
    ticks = [gc.peek_next(i) - 1 for i in range(N_PROCS)]
    live = [i for i in range(N_PROCS) if ticks[i] > 0]
    sp_clock = wait_clock.engine_clocks[PROC_NAME_TO_IDX["SP"]]
    for p in live:
        vc = bass_rust.VectorClock()
        vc.require_at_least(p, ticks[p])
        nop = nc.sync.nop(nofuse=True, hint="tail_wait_funnel")
        wait_clock.add_sem_waits(
            nop.ins, bass_rust.ScopedClock({None: vc}), cur_clock=sp_clock
        )
        sp_clock.require_at_least(None, p, ticks[p])
    drain_inst = nc.sync.drain()
    wait_clock.add_sem_waits(
        drain_inst.ins, bass_rust.ScopedClock({None: gc}), cur_clock=sp_clock
    )
    nc.all_engine_barrier()
    assert self.sems is not None
    popped = nc._tile_sem_poison_stack.pop()
    assert popped is self._sem_poison
    nc.clear_and_free_semaphores(list(self.sems.allocated().values()))
    nc.all_engine_barrier()


tile.TileContext._drain_and_barrier = _patched_drain_and_barrier


def fix_excess_waits(nc, max_waits=1):
    """Hoist excess per-instruction sync-waits onto injected same-engine
    NoOps placed immediately before the offender (engine streams run in
    basic-block order)."""
    for bb in nc.m.functions[0].blocks:
        insts = bb.instructions
        if not any(
            i.sync_info and i.sync_info.on_wait and len(i.sync_info.on_wait) > max_waits
            for i in insts
        ):
            continue
        out = []
        for inst in insts:
            si = inst.sync_info
            if si and si.on_wait and len(si.on_wait) > max_waits:
                waits = list(si.on_wait)
                extra, keep = waits[:-max_waits], waits[-max_waits:]
                for i in range(0, len(extra), max_waits):
                    chunk = extra[i : i + max_waits]
                    nop = mybir.InstNoOp(
                        name=nc.get_next_instruction_name(), ins=[], outs=[]
                    )
                    nop.engine = inst.engine
                    nop.sync_info = bass_rust.SyncInfo(on_wait=chunk, on_update=[])
                    nc.register_instruction(nop)
                    out.append(nop)
                si.on_wait = keep
            out.append(inst)
        bb.instructions = out


def dedup_ldweights(nc):
    """Drop an InstLdweights whose weights AP (and modes) match the previous
    weight load on the PE stream — the PE array keeps the stationary operand
    across matmuls, so a reload of identical weights only burns LDW cycles.
    Only loads carrying no sem waits/updates are removed."""
    import concourse.mybir as mb

    for bb in nc.m.functions[0].blocks:
        last = None
        out = []
        for inst in bb.instructions:
            if isinstance(inst, mb.InstLdweights):
                s = (
                    str(inst.ins[0]),
                    str(getattr(inst, "perf_mode", None)),
                    str(getattr(inst, "is_transpose", None)),
                    str(getattr(inst, "tile_position", None)),
                )
                clean = not inst.sync_info or (
                    not inst.sync_info.on_wait and not inst.sync_info.on_update
                )
                if s == last and clean:
                    continue
                last = s
            out.append(inst)
        bb.instructions = out


# ------------------------------------------------------------- kernel build
def build_kernel(reps=1, variant="full", heads=H, unroll=False):
    nc = bass.Bass("TRN2", target_bir_lowering=False, debug=False)

    x_d = nc.dram_tensor("x", [IPC, C, HW], F16, kind="ExternalInput")
    wv8_d = nc.dram_tensor("wv8", [C, C], F8, kind="ExternalInput")
    w2_d = nc.dram_tensor("w2T", [C, OUT], F16, kind="ExternalInput")
    wc8_d = nc.dram_tensor("wc8", [C, OUT], F8, kind="ExternalInput")
    vpos_d = nc.dram_tensor("vposT", [128, KC * N], F32, kind="ExternalInput")
    maskT_d = nc.dram_tensor("maskT", [128, KC * heads], F8, kind="ExternalInput")
    mask2_d = nc.dram_tensor("mask2", [heads, KC * 128], F16, kind="ExternalInput")
    oneh_d = nc.dram_tensor("oneh", [128, NT * 128], F16, kind="ExternalInput")
    pcm_d = nc.dram_tensor("pcm", [128, OUT], F16, kind="ExternalInput")
    # x-path rows land contiguously (token-major, no token-0 gaps); host
    # reassembles [IPC, N, OUT] from outx + out0.
    outx_d = nc.dram_tensor("outx", [XTOK, OUT], F16, kind="ExternalOutput")
    out0_d = nc.dram_tensor("out0", [IPC, OUT], F16, kind="ExternalOutput")

    with tile.TileContext(nc) as tc, ExitStack() as ctx:
        wv_pool = ctx.enter_context(tc.tile_pool(name="wv", bufs=1))
        w2_pool = ctx.enter_context(tc.tile_pool(name="w2", bufs=1))
        wc_pool = ctx.enter_context(tc.tile_pool(name="wc", bufs=1))
        cpool = ctx.enter_context(tc.tile_pool(name="consts", bufs=1))
        xpool = ctx.enter_context(tc.tile_pool(name="xT", bufs=1))
        spool = ctx.enter_context(tc.tile_pool(name="small", bufs=2))
        tpool = ctx.enter_context(tc.tile_pool(name="tT8", bufs=2))
        vpool = ctx.enter_context(tc.tile_pool(name="vT", bufs=2))
        apool = ctx.enter_context(tc.tile_pool(name="attn", bufs=2))
        ppool = ctx.enter_context(tc.tile_pool(name="pp8", bufs=1))
        opool = ctx.enter_context(tc.tile_pool(name="outsb", bufs=2))
        upool = ctx.enter_context(tc.tile_pool(name="uT", bufs=1))
        pv = ctx.enter_context(tc.tile_pool(name="pv", bufs=2, space="PSUM"))
        pS = ctx.enter_context(tc.tile_pool(name="pS", bufs=1, space="PSUM"))
        pA = ctx.enter_context(tc.tile_pool(name="pA", bufs=3, space="PSUM"))
        po = ctx.enter_context(tc.tile_pool(name="po", bufs=2, space="PSUM"))

        # ---- resident weights/constants (loaded outside the rep loop)
        wv8_sb = wv_pool.tile([128, KC * C], F8, name="wv8")
        for kc in range(KC):
            nc.sync.dma_start(
                wv8_sb[:, kc * C : (kc + 1) * C],
                wv8_d.ap()[kc * 128 : (kc + 1) * 128, :],
            )
        w2_sb, wc8_sb = [], []
        for kc in range(KC):
            w = w2_pool.tile([128, OUT], F16, name=f"w2{kc}", tag=f"w2{kc}")
            nc.sync.dma_start(w[:], w2_d.ap()[kc * 128 : (kc + 1) * 128, :])
            w2_sb.append(w)
            w8 = wc_pool.tile([128, OUT], F8, name=f"wc{kc}", tag=f"wc{kc}")
            nc.sync.dma_start(w8[:], wc8_d.ap()[kc * 128 : (kc + 1) * 128, :])
            wc8_sb.append(w8)
        vpos_sb = cpool.tile([128, KC * N], F32, name="vpos")
        nc.sync.dma_start(vpos_sb[:], vpos_d.ap())
        maskT_sb = cpool.tile([128, KC * heads], F8, name="maskT")
        nc.sync.dma_start(maskT_sb[:], maskT_d.ap())
        mask2_sb = cpool.tile([heads, KC * 128], F16, name="mask2")
        nc.sync.dma_start(mask2_sb[:], mask2_d.ap())
        oneh_sb = cpool.tile([128, NT * 128], F16, name="oneh")
        nc.sync.dma_start(oneh_sb[:], oneh_d.ap())
        pcm_sb = cpool.tile([128, OUT], F16, name="pcm")
        nc.sync.dma_start(pcm_sb[:], pcm_d.ap())

        # x tokens resident in [channel, kc-major global token] layout:
        # xT[p, kc, j] = x[item j//49, kc*128+p, j%49], fp16
        xT_sb = xpool.tile([128, KC * XTOK], F16, name="xTall")
        # uT[p, kc, i] = SU * ctx[item i, kc*128+p], fp8
        uT_sb = upool.tile([128, KC * IPC], F8, name="uT")

        def work():
            body(nc, tc, x_d, (outx_d, out0_d), wv8_sb, w2_sb, wc8_sb, vpos_sb,
                 maskT_sb, mask2_sb, oneh_sb, pcm_sb, xT_sb, uT_sb,
                 spool, tpool, vpool, apool, ppool, opool, pv, pS, pA, po,
                 variant, heads)

        if reps == 1:
            work()
        elif unroll:
            for _ in range(reps):
                work()
        else:
            with tc.For_i(0, reps, 1):
                work()

    dedup_ldweights(nc)
    fix_excess_waits(nc)
    return nc


def body(nc, tc, x_d, outs, wv8_sb, w2_sb, wc8_sb, vpos_sb, maskT_sb,
         mask2_sb, oneh_sb, pcm_sb, xT_sb, uT_sb, spool, tpool, vpool,
         apool, ppool, opool, pv, pS, pA, po, variant="full", heads=H):
    outx_d, out0_d = outs
    scale_exp = float((C // heads) ** -0.5)
    wv8_v = wv8_sb[:].rearrange("p (k c) -> p k c", k=KC)
    xT_v = xT_sb[:].rearrange("p (k j) -> p k j", k=KC)
    uT_v = uT_sb[:].rearrange("p (k i) -> p k i", k=KC)
    vpos3 = vpos_sb[:].rearrange("p (k n) -> p k n", k=KC)

    def build_tT8(g):
        # tT8 layout: [128, KC*(GI*N)] fp8 = 32*t, kc-major so the DoubleRow
        # moving operand spans 2 adjacent kc subtiles: [p, 2, 400]
        tT8 = tpool.tile([128, KC * GI * N], F8, name="tT8", tag="tT8")
        t4 = tT8[:].rearrange("p (k i n) -> p k i n", k=KC, i=GI)
        for it in range(GI):
            gi = g * GI + it
            dst = xT_v[:, :, gi * HW : (gi + 1) * HW]
            nc.sync.dma_start(
                dst, x_d.ap()[gi].rearrange("(k p) n -> p k n", p=128)
            )
            # spatial tokens: fp8(32 * x)
            nc.scalar.activation(
                t4[:, :, it, 1:N], dst,
                mybir.ActivationFunctionType.Copy, scale=32.0,
            )
            # mean token: fp8(32/49 * sum_s x)
            xsum = spool.tile([128, KC], F32, name="xsum", tag="xsum")
            nc.vector.reduce_sum(xsum[:], dst, axis=mybir.AxisListType.X)
            nc.scalar.activation(
                t4[:, :, it, 0], xsum[:],
                mybir.ActivationFunctionType.Copy, scale=32.0 / HW,
            )
        return tT8

    def vproj(g, tT8):
        # vT' = SV * v fp16 via fp8 DoubleRow matmuls (contraction 256/chunk).
        # Per jb, also emit the attention prework on DVE/Act so attnS's
        # matmuls have no cross-engine wait: v0s slice + fp8 pprod = v * v0.
        tT8_3 = tT8[:].rearrange("p (k m) -> p k m", k=KC)
        vTg = vpool.tile([128, JC * NG], F16, name="vTg", tag="vTg")
        v4 = vTg[:].rearrange("p (j i n) -> p j i n", j=JC, i=GI)
        v0s = spool.tile([128, JC * GI], F16, name="v0s", tag="v0s")
        v0s4 = v0s[:].rearrange("p (j i o) -> p j i o", j=JC, o=1)
        pp8 = ppool.tile([128, KC * NG], F8, name="pp8", tag="pp8")
        pp8_4 = pp8[:].rearrange("p (j i n) -> p j i n", j=JC, i=GI)
        for jb in range(JC):
            psum = pv.tile([128, NG], F32, name="pvt", tag="pvt")
            for k2 in range(KC2):
                nc.tensor.matmul(
                    psum[:],
                    wv8_v[:, 2 * k2 : 2 * k2 + 2, jb * 128 : (jb + 1) * 128],
                    tT8_3[:, 2 * k2 : 2 * k2 + 2, :],
                    start=(k2 == 0),
                    stop=(k2 == KC2 - 1),
                    perf_mode=mybir.MatmulPerfMode.DoubleRow,
                )
            nc.vector.tensor_add(
                v4[:, jb],
                psum[:].rearrange("p (i n) -> p i n", i=GI),
                vpos3[:, jb : jb + 1, :].broadcast_to((128, GI, N)),
            )
            # v0s = v0/32 (fp16-normal range)
            nc.scalar.activation(
                v0s4[:, jb, :, 0], v4[:, jb, :, 0],
                mybir.ActivationFunctionType.Copy, scale=2.0 ** -16,
            )
            # pprod = (vT'/64) * (v0/32) = v * v0, fp8
            nc.vector.scalar_tensor_tensor(
                pp8_4[:, jb],
                v4[:, jb],
                2.0 ** -6,
                v0s4[:, jb].broadcast_to((128, GI, N)),
                op0=mybir.AluOpType.mult,
                op1=mybir.AluOpType.mult,
            )
        return vTg, pp8

    def attnS(g, vTg, pp8):
        # S per head: fp8 DoubleRow masked matmuls over pprod
        maskT_v = maskT_sb[:].rearrange("p (k h) -> p k h", k=KC)
        pp8_3 = pp8[:].rearrange("p (k m) -> p k m", k=KC)
        psum_S = pS.tile([heads, NG], F32, name="psS", tag="psS")
        for k2 in range(KC2):
            nc.tensor.matmul(
                psum_S[:],
                maskT_v[:, 2 * k2 : 2 * k2 + 2, :],
                pp8_3[:, 2 * k2 : 2 * k2 + 2, :],
                start=(k2 == 0),
                stop=(k2 == KC2 - 1),
                perf_mode=mybir.MatmulPerfMode.DoubleRow,
            )
        return psum_S

    xp_ps = {}

    def xpath_part(ti, part):
        # half of one token tile's contraction (kcs 0-7 or 8-15), both
        # out-column halves; kc-outer so each stationary x-token chunk is
        # loaded once per pair of matmuls. The accumulation group stays open
        # across interleaved matmuls to other PSUM banks (psA), giving the
        # scheduler ~3.6us PE filler units. part 1 finishes with the pconst
        # one-hot matmul, Act-engine copies, and the whole-tile DMA.
        m0 = ti * 128
        mw = min(128, XTOK - m0)
        if part == 0:
            xp_ps[ti] = [po.tile([128, 512], F32, name=f"pso{oc}", tag="pso")
                         for oc in range(OC2)]
        ps = xp_ps[ti]
        for kc in range(8 * part, 8 * part + 8):
            for oc in range(OC2):
                nc.tensor.matmul(
                    ps[oc][:mw, :],
                    xT_v[:, kc, m0 : m0 + mw],
                    w2_sb[kc][:, oc * 512 : (oc + 1) * 512],
                    start=(kc == 0),
                    stop=False,
                    skip_group_check=True,
                )
        if part == 1:
            for oc in range(OC2):
                nc.tensor.matmul(
                    ps[oc][:mw, :],
                    oneh_sb[:, m0 : m0 + mw],
                    pcm_sb[:, oc * 512 : (oc + 1) * 512],
                    start=False,
                    stop=True,
                    skip_group_check=True,
                )
            osb = opool.tile([128, OUT], F16, name="osb", tag="osb")
            for oc in range(OC2):
                nc.scalar.activation(
                    osb[:mw, oc * 512 : (oc + 1) * 512], ps[oc][:mw, :],
                    mybir.ActivationFunctionType.Copy,
                )
            nc.sync.dma_start(outx_d.ap()[m0 : m0 + mw, :], osb[:mw, :])
            del xp_ps[ti]

    def xpath_tile(ti):
        xpath_part(ti, 0)
        xpath_part(ti, 1)

    def attnAV(g, vTg, psum_S, fillers=()):
        # A = softmax(S); u' = SV * sum_m A[h(c), m] v[c, m] -> uT fp8.
        # fillers: x-path oc-blocks interleaved so the PE streams matmuls
        # while the DVE drains each psA bank.
        v3 = vTg[:].rearrange("p (j m) -> p j m", j=JC)
        e_sb = apool.tile([heads, NG], F32, name="esb", tag="esb")
        nc.scalar.activation(
            e_sb[:], psum_S[:], mybir.ActivationFunctionType.Exp,
            scale=scale_exp,
        )
        d_sb = apool.tile([heads, GI], F32, name="dsb", tag="dsb")
        nc.vector.reduce_sum(
            d_sb[:],
            e_sb[:].rearrange("p (i n) -> p i n", i=GI),
            axis=mybir.AxisListType.X,
        )
        r_sb = apool.tile([heads, GI], F32, name="rsb", tag="rsb")
        nc.vector.reciprocal(r_sb[:], d_sb[:])
        a_sb = apool.tile([heads, NG], F16, name="asb", tag="asb")
        nc.vector.tensor_mul(
            a_sb[:].rearrange("p (i n) -> p i n", i=GI),
            e_sb[:].rearrange("p (i n) -> p i n", i=GI),
            r_sb[:].rearrange("p (i o) -> p i o", o=1).broadcast_to((heads, GI, N)),
        )
        fillers = list(fillers)
        chunks = [range(0, 3), range(3, 6), range(6, 9), range(9, 12),
                  range(12, 15), range(15, 16)]
        for ci, chunk in enumerate(chunks):
            for jb in chunk:
                psum_a = pA.tile([128, NG], F32, name="psA", tag="psA")
                nc.tensor.matmul(
                    psum_a[:],
                    mask2_sb[:, jb * 128 : (jb + 1) * 128],
                    a_sb[:],
                    start=True,
                    stop=True,
                )
                p2 = apool.tile([128, NG], F16, name="p2", tag="p2")
                nc.vector.tensor_mul(p2[:], psum_a[:], v3[:, jb])
                ctx8 = apool.tile([128, GI], F32, name="ctx8", tag="ctx8")
                nc.vector.reduce_sum(
                    ctx8[:],
                    p2[:].rearrange("p (i n) -> p i n", i=GI),
                    axis=mybir.AxisListType.X,
                )
                nc.scalar.activation(
                    uT_v[:, jb, g * GI : (g + 1) * GI], ctx8[:],
                    mybir.ActivationFunctionType.Copy, scale=SU / SV,
                )
            if ci < len(fillers):
                xpath_part(*fillers[ci])
        for fi in range(len(chunks), len(fillers)):
            xpath_part(*fillers[fi])

    def out0proj():
        # out0 = u @ Wc.T: psum = (SU*ctx)@(SW*Wc) -> scale 1/(SU*SW)
        o0 = opool.tile([IPC, OUT], F16, name="o0sb", tag="o0sb")
        for oc in range(OC2):
            psum = po.tile([128, 512], F32, name="ps0", tag="pso")
            for kc in range(KC):
                nc.tensor.matmul(
                    psum[:IPC, :],
                    uT_v[:, kc, :],
                    wc8_sb[kc][:, oc * 512 : (oc + 1) * 512],
                    start=(kc == 0),
                    stop=(kc == KC - 1),
                )
            nc.scalar.activation(
                o0[:, oc * 512 : (oc + 1) * 512], psum[:IPC, :],
                mybir.ActivationFunctionType.Copy, scale=1.0 / (SU * SW),
            )
        nc.sync.dma_start(out0_d.ap()[:], o0[:])

    # ---- schedule: fp8 vproj / attention pipelined with fp16 x-path tiles
    if variant == "full":
        vt0 = vproj(0, build_tT8(0))
        s_prev = attnS(0, *vt0)
        v_prev = vt0[0]
        for g in range(1, G + 1):
            if g < G:
                vtg = vproj(g, build_tT8(g))
            tiles = [3 * (g - 1) + t for t in range(3)]
            if g == G:
                tiles.append(12)
            fillers = [(t, p) for t in tiles for p in range(2)]
            attnAV(g - 1, v_prev, s_prev, fillers)
            if g < G:
                s_prev = attnS(g, *vtg)
                v_prev = vtg[0]
        out0proj()
    elif variant == "vproj":
        for g in range(G):
            vproj(g, build_tT8(g))
    elif variant in ("xpath", "xmm", "xnodma"):
        for it in range(IPC):
            nc.sync.dma_start(
                xT_v[:, :, it * HW : (it + 1) * HW],
                x_d.ap()[it].rearrange("(k p) n -> p k n", p=128),
            )
        if variant == "xmm":
            # sim bisect: matmuls only, no psum drain
            for ti in range(NT):
                for oc in range(OC2):
                    m0 = ti * 128
                    mw = min(128, XTOK - m0)
                    psum = po.tile([128, 512], F32, name="pso", tag="pso")
                    for kc in range(KC):
                        nc.tensor.matmul(
                            psum[:mw, :],
                            xT_v[:, kc, m0 : m0 + mw],
                            w2_sb[kc][:, oc * 512 : (oc + 1) * 512],
                            start=(kc == 0),
                            stop=(kc == KC - 1),
                        )
        elif variant == "xnodma":
            # sim bisect: matmuls + copies, no out DMA
            for ti in range(NT):
                for oc in range(OC2):
                    m0 = ti * 128
                    mw = min(128, XTOK - m0)
                    psum = po.tile([128, 512], F32, name="pso", tag="pso")
                    for kc in range(KC):
                        nc.tensor.matmul(
                            psum[:mw, :],
                            xT_v[:, kc, m0 : m0 + mw],
                            w2_sb[kc][:, oc * 512 : (oc + 1) * 512],
                            start=(kc == 0),
                            stop=(kc == KC - 1),
                        )
                    osb = opool.tile([128, OUT], F16, name="osb", tag="osb")
                    nc.scalar.activation(
                        osb[:mw, oc * 512 : (oc + 1) * 512], psum[:mw, :],
                        mybir.ActivationFunctionType.Copy,
                    )
        else:
            for ti in range(NT):
                xpath_tile(ti)
    elif variant == "attn":
        for g in range(G):
            vTg, v0s = vproj(g, build_tT8(g))
            attnAV(g, vTg, attnS(g, vTg, v0s))
        out0proj()


_NC_CACHE = {}
_RUN_CACHE = {}


def _get_nc(heads):
    if heads not in _NC_CACHE:
        _NC_CACHE[heads] = build_kernel(heads=heads)
    return _NC_CACHE[heads]


def _run(nc, in_maps):
    """run_bass_kernel_spmd equivalent (axon/PJRT path) with: the jitted
    executable cached across calls, weight-like inputs passed replicated
    (uploaded once, not 8x), and donated output buffers created on device
    (no zero upload)."""
    import jax
    import jax.numpy as jnp
    import numpy as _np
    from jax.sharding import Mesh, PartitionSpec, NamedSharding
    from jax.experimental.shard_map import shard_map
    import concourse.mybir as mb
    from concourse import bass2jax as b2j

    # inputs where every core got the identical array object -> replicated
    replicated = {
        nm
        for nm in in_maps[0]
        if all(m[nm] is in_maps[0][nm] for m in in_maps)
    }

    key = id(nc)
    if key not in _RUN_CACHE:
        b2j.install_neuronx_cc_hook()
        in_names, out_names, out_avals = [], [], []
        partition_name = (
            nc.partition_id_tensor.name if nc.partition_id_tensor else None
        )
        for alloc in nc.m.functions[0].allocations:
            if not isinstance(alloc, mb.MemoryLocationSet):
                continue
            name = alloc.memorylocations[0].name
            if alloc.kind == "ExternalInput":
                if name != partition_name:
                    in_names.append(name)
            elif alloc.kind == "ExternalOutput":
                shape = tuple(alloc.tensor_shape)
                dtype = mb.dt.np(alloc.dtype)
                out_names.append(name)
                out_avals.append(jax.core.ShapedArray(shape, dtype))
        n_params = len(in_names)
        n_outs = len(out_avals)
        all_names = list(in_names) + list(out_names)
        if partition_name is not None:
            all_names.append(partition_name)
        donate = tuple(range(n_params, n_params + n_outs))

        def _body(*args):
            operands = list(args)
            if partition_name is not None:
                operands.append(b2j.partition_id_tensor())
            outs = b2j._bass_exec_p.bind(
                *operands,
                out_avals=tuple(out_avals),
                in_names=tuple(all_names),
                out_names=tuple(out_names),
                lowering_input_output_aliases=(),
                sim_require_finite=True,
                sim_require_nnan=True,
                nc=nc,
            )
            return tuple(outs)

        devices = jax.devices()[:CORES]
        mesh = Mesh(_np.asarray(devices), ("core",))
        in_specs = tuple(
            PartitionSpec() if nm in replicated else PartitionSpec("core")
            for nm in in_names
        ) + (PartitionSpec("core"),) * n_outs
        out_specs = (PartitionSpec("core"),) * n_outs
        sharded = jax.jit(
            shard_map(
                _body, mesh=mesh, in_specs=in_specs, out_specs=out_specs,
                check_rep=False,
            ),
            donate_argnums=donate,
            keep_unused=True,
        )
        zeros_fns = [
            jax.jit(
                (lambda shape, dtype: lambda: jnp.zeros(shape, dtype))(
                    (CORES * av.shape[0], *av.shape[1:]), av.dtype
                ),
                out_shardings=NamedSharding(mesh, PartitionSpec("core")),
            )
            for av in out_avals
        ]
        _RUN_CACHE[key] = (
            sharded, in_names, out_names, out_avals, zeros_fns, replicated
        )

    sharded, in_names, out_names, out_avals, zeros_fns, replicated_c = (
        _RUN_CACHE[key]
    )
    assert replicated == replicated_c, "replication pattern changed"
    args = [
        _np.asarray(in_maps[0][nm])
        if nm in replicated
        else _np.concatenate([_np.asarray(m[nm]) for m in in_maps], axis=0)
        for nm in in_names
    ]
    dev_zeros = [f() for f in zeros_fns]
    out_arrs = sharded(*args, *dev_zeros)
    return [
        {
            nm: _np.asarray(out_arrs[i]).reshape(CORES, *out_avals[i].shape)[c]
            for i, nm in enumerate(out_names)
        }
        for c in range(CORES)
    ]


# ---------------------------------------------------------------- host side
def _fp8(a):
    f8np = mybir.dt.np(F8)  # ml_dtypes.float8_e4m3 (TRN range, max 240)
    return np.clip(a, -240.0, 240.0).astype(f8np)


def make_in_maps(inputs, heads=H):
    x = np.asarray(inputs["x"], np.float32)
    pos_emb = np.asarray(inputs["pos_emb"], np.float32)
    Wv = np.asarray(inputs["Wv"], np.float32)
    bv = np.asarray(inputs["bv"], np.float32)
    Wc = np.asarray(inputs["Wc"], np.float32)
    bc = np.asarray(inputs["bc"], np.float32)
    num_heads = int(np.asarray(inputs["num_heads"]))
    assert num_heads == heads and x.shape == (B, C, S, S)
    assert 1 <= heads <= 128 and C % heads == 0

    wv8 = _fp8(64.0 * Wv.T)                       # [C(k), C(c)]
    W2 = Wc @ Wv                                  # [OUT, C]
    w2T = np.ascontiguousarray(W2.T).astype(np.float16)   # [C, OUT]
    wc8 = _fp8(SW * Wc.T)                         # [C, OUT]

    # vposT[128, kc*50 + n] = SV * (pos_emb @ Wv.T + bv).T chunk-tiled
    vpos = SV * (pos_emb @ Wv.T + bv).astype(np.float32)  # [N, C]
    vposT = np.empty((128, KC * N), np.float32)
    for kc in range(KC):
        vposT[:, kc * N : (kc + 1) * N] = vpos[:, kc * 128 : (kc + 1) * 128].T

    # maskT[p, kc*heads + h] = 1 if channel kc*128+p belongs to head h
    head_of = np.arange(C) // (C // heads)
    maskT = np.zeros((128, KC * heads), mybir.dt.np(F8))
    mask2 = np.zeros((heads, KC * 128), np.float16)
    for kc in range(KC):
        for p in range(128):
            h = head_of[kc * 128 + p]
            maskT[p, kc * heads + h] = 1.0
            mask2[h, kc * 128 + p] = 1.0

    # x-path pos constant: out[n>=1] += pconst[n] via one-hot matmul
    # oneh[p, j] = 1 iff p == j % 49; pcm[p] = pconst[p+1]
    oneh = np.zeros((128, NT * 128), np.float16)
    j = np.arange(XTOK)
    oneh[j % HW, j] = 1.0
    pconst = pos_emb @ W2.T + bv @ Wc.T           # [N, OUT]
    pcm = np.zeros((128, OUT), np.float16)
    pcm[:HW] = pconst[1:].astype(np.float16)

    xr16 = np.ascontiguousarray(x.reshape(B, C, HW).astype(np.float16))
    in_maps = []
    for core in range(CORES):
        in_maps.append(
            {
                "x": xr16[core * IPC : (core + 1) * IPC],
                "wv8": wv8,
                "w2T": w2T,
                "wc8": wc8,
                "vposT": vposT,
                "maskT": maskT,
                "mask2": mask2,
                "oneh": oneh,
                "pcm": pcm,
            }
        )

    return in_maps


def kernel(**inputs):
    from concourse._compat import axon_active

    heads = int(np.asarray(inputs["num_heads"]))
    in_maps = make_in_maps(inputs, heads)
    nc = _get_nc(heads)
    if axon_active():
        results = _run(nc, in_maps)
    else:
        results = run_bass_kernel_spmd(nc, in_maps, list(range(CORES))).results
    out = np.empty((B, N, OUT), np.float32)
    for i in range(CORES):
        blk = out[i * IPC : (i + 1) * IPC]
        blk[:, 1:] = np.asarray(results[i]["outx"]).reshape(IPC, HW, OUT)
        blk[:, 0] = np.asarray(results[i]["out0"])
    bc = np.asarray(inputs["bc"], np.float32)
    if bc.any():
        out = out + bc[None, None, :]
    return out
